# revision 1
# baseline (speedup 1.0000x reference)
"""Multi-head causal self-attention on 8 TRN2 NeuronCores.

Problem (nn_MultiHeadAttention): B=2, T=2048, C=1024, H=16 heads, hs=64.
  q,k,v = per-head projections of x; causal softmax(q k^T / 8) v;
  concat heads; out = att @ Wo + bo.

Sharding: core c in 0..7 -> (batch b = c//4, head-group g = c%4, 4 heads each).
Each core computes Q/K/V + flash-style causal attention for its 4 heads on its
batch, normalized attention outputs are AllGathered across the 4 cores of the
same batch (replica groups [0-3], [4-7]), then each core computes a disjoint
256-column slice of the output projection (column-parallel Wo) + bias slice.
Host does a pure concat of the 8 disjoint output slices.

All matmuls run as float32r (single-pass fp32 PE mode, 4x faster than fp32).
Attention works in transposed layout throughout: Q^T/K^T [d, t], scores
S^T [s, t], P^T = exp(S^T/8) with causal mask, att^T [d, t] via
lhsT=[V_h | ones] (row 64 of the PSUM accumulator = softmax denominator).
Normalization multiplies by a PE-broadcast reciprocal row.

Scheduling notes: per-engine instruction order is static, so projection
(stage 1) and output-projection (stage 3) work is interleaved into the
attention head loops to fill PE bubbles left by the scores->exp->AV chain,
and stage-3 matmuls for t-block qb are emitted only during stage-2 of qb+1,
when their AllGathered inputs have already landed.
"""

import numpy as np
from contextlib import ExitStack

import concourse.bass as bass
import concourse.mybir as mybir
import concourse.tile as tile
from concourse import bacc
from concourse.bass_utils import run_bass_kernel_spmd

F32 = mybir.dt.float32
F32R = mybir.dt.float32r
EXP = mybir.ActivationFunctionType.Exp

N_CORES = 8
B = 2
T = 2048
C = 1024
NH = 16
HS = 64
E = 1024
GROUPS = 4          # head groups (tensor-parallel ranks per batch)
HPG = NH // GROUPS  # 4 heads per core
ES = E // GROUPS    # 256 output columns per core
HD = HPG * HS       # 256 local attention-output rows

P = 128             # partition tile
TBLK = 512          # t-block (matmul moving dim)
NTB = T // TBLK     # 4
NCT = C // P        # 8 contraction tiles for projections
NST = T // P        # 16 key tiles
VW = HS + 1         # V lhsT width per head (64 V cols + ones col)

REPLICA_GROUPS = [[0, 1, 2, 3], [4, 5, 6, 7]]


def build_nc(with_collective=True):
    """Build + compile the per-core SPMD program. Same program on all cores."""
    nc = bacc.Bacc(
        "TRN2", target_bir_lowering=False, debug=False, num_devices=N_CORES
    )

    xT = nc.dram_tensor("xT", [C, T], F32R, kind="ExternalInput").ap()
    wq = nc.dram_tensor("wq", [C, HD], F32R, kind="ExternalInput").ap()
    wk = nc.dram_tensor("wk", [C, HD], F32R, kind="ExternalInput").ap()
    wv = nc.dram_tensor("wv", [C, HD], F32R, kind="ExternalInput").ap()
    wo = nc.dram_tensor("wo", [E, ES], F32R, kind="ExternalInput").ap()
    bo = nc.dram_tensor("bo", [1, ES], F32R, kind="ExternalInput").ap()
    tri = nc.dram_tensor("tri", [P, P], F32R, kind="ExternalInput").ap()
    onesc = nc.dram_tensor("onesc", [1, P], F32R, kind="ExternalInput").ap()
    vones = nc.dram_tensor("vones", [P, HPG], F32R, kind="ExternalInput").ap()
    out = nc.dram_tensor("out", [T, ES], F32, kind="ExternalOutput").ap()

    with tile.TileContext(nc) as tc, ExitStack() as ctx:
        wp = ctx.enter_context(tc.tile_pool(name="wp", bufs=1))
        xp = ctx.enter_context(tc.tile_pool(name="xp", bufs=2))
        qkp = ctx.enter_context(tc.tile_pool(name="qkp", bufs=1))
        vp = ctx.enter_context(tc.tile_pool(name="vp", bufs=1))
        ptp = ctx.enter_context(tc.tile_pool(name="ptp", bufs=6))
        attp = ctx.enter_context(tc.tile_pool(name="attp", bufs=2))
        smp = ctx.enter_context(tc.tile_pool(name="smp", bufs=4))
        outp = ctx.enter_context(tc.tile_pool(name="outp", bufs=3))
        lhp = ctx.enter_context(tc.tile_pool(name="lhp", bufs=16))
        # PSUM: 8 banks total.  st2 [128,1024] = 2 banks x 2 bufs = 4,
        # attv 1 bank x 2, small (bc / out-proj) 1 bank x 2.
        ps2 = ctx.enter_context(tc.tile_pool(name="ps2", bufs=2, space="PSUM"))
        psB = ctx.enter_context(tc.tile_pool(name="psB", bufs=2, space="PSUM"))
        psC = ctx.enter_context(tc.tile_pool(name="psC", bufs=2, space="PSUM"))
        dramp = ctx.enter_context(tc.tile_pool(name="dramp", bufs=1, space="DRAM"))

        # ---- small constants ----
        ones = wp.tile([1, P], F32R, tag="ones")
        nc.sync.dma_start(ones[:], onesc[:])
        tri_sb = wp.tile([P, P], F32R, tag="tri")
        nc.sync.dma_start(tri_sb[:], tri[:])
        bias_sb = wp.tile([1, ES], F32R, tag="bias")

        w_sb = {n: [] for n in ("wq", "wk", "wv", "wo")}
        for name in ("wq", "wk", "wv", "wo"):
            for ci in range(NCT):
                w_sb[name].append(
                    wp.tile([P, ES], F32R, tag=f"{name}{ci}", name=f"{name}{ci}")
                )

        # x^T tiles per (c-tile, t-block), double-buffered across t-blocks:
        # x(tb) is only read by stage-1(tb), so two t-blocks' worth suffices
        xt_of = {}

        def alloc_xt(tb):
            xt_of[tb] = [
                xp.tile([P, TBLK], F32R, tag=f"x{ci}", name=f"x{ci}_{tb}")
                for ci in range(NCT)
            ]
            return xt_of[tb]

        # merged Q^T/K^T per head pair: col = tb*1024 + qk*512 + t_local
        # (pair p holds heads 2p (rows 0-63) and 2p+1 (rows 64-127))
        qkt = [qkp.tile([P, 2 * T], F32R, tag=f"qk{p_}", name=f"qk{p_}")
               for p_ in range(2)]

        def qt_slice(pr, r0, rn, t0, tn):
            tb, tl = t0 // TBLK, t0 % TBLK
            base = tb * 1024 + tl
            return qkt[pr][r0:r0 + rn, base:base + tn]

        def kt_slice(pr, r0, rn, s0, sn):
            tb, sl = s0 // TBLK, s0 % TBLK
            base = tb * 1024 + TBLK + sl
            return qkt[pr][r0:r0 + rn, base:base + sn]

        v_sb = [vp.tile([P, HPG * VW], F32R, tag=f"v{st}", name=f"v{st}")
                for st in range(NST)]

        # ---------------- stage-1 pieces ----------------
        def emit_x_dma(tb):
            ts_ = tb * TBLK
            xt = alloc_xt(tb)
            for ci in range(NCT):
                nc.sync.dma_start(
                    xt[ci][:], xT[ci * P:(ci + 1) * P, ts_:ts_ + TBLK])

        def emit_qk_proj(tb, pr, which):
            # one [128,512] accumulation on the psC "small" tag (see
            # emit_v_proj for why not st2); which=0 -> Q, which=1 -> K
            xt = xt_of[tb]
            wn = "wq" if which == 0 else "wk"
            ps = psC.tile([P, TBLK], F32, tag="small",
                          name=f"qkps{tb}_{pr}_{which}")
            for ci in range(NCT):
                nc.tensor.matmul(
                    ps[:],
                    lhsT=w_sb[wn][ci][:, pr * P:(pr + 1) * P],
                    rhs=xt[ci][:],
                    start=(ci == 0), stop=(ci == NCT - 1),
                )
            base = tb * 1024 + which * TBLK
            nc.vector.tensor_copy(qkt[pr][:, base:base + TBLK], ps[:])

        def emit_v_proj(st):
            # psC "small" tag, NOT ps2: a V filler holding an st2 slot for its
            # 8-matmul group would degrade the scores/exp pipeline to
            # single-buffering
            tb, sl = st // 4, (st % 4) * P
            xt = xt_of[tb]
            vps = psC.tile([P, TBLK], F32, tag="small", name=f"vps{st}")
            for ci in range(NCT):
                nc.tensor.matmul(
                    vps[:, 0:HD],
                    lhsT=xt[ci][:, sl:sl + P],
                    rhs=w_sb["wv"][ci][:],
                    start=(ci == 0), stop=(ci == NCT - 1),
                )
            nc.sync.dma_start(
                v_sb[st][:].rearrange("p (h x) -> p h x", h=HPG)[:, :, HS:VW],
                vones[:].rearrange("p (h o) -> p h o", o=1),
            )
            nc.vector.tensor_copy(
                v_sb[st][:].rearrange("p (h x) -> p h x", h=HPG)[:, :, 0:HS],
                vps[:, 0:HD].rearrange("p (h x) -> p h x", h=HPG),
            )

        def qk_chunks(tb):
            return [lambda tb=tb, pr=pr, w=w: emit_qk_proj(tb, pr, w)
                    for pr in range(2) for w in range(2)]

        def v_chunks(tb):
            return [lambda st=st: emit_v_proj(st)
                    for st in range(4 * tb, 4 * tb + 4)]

        # ------- stage-2 piece (one head PAIR of one t-block, jointly) ------
        def emit_headpair(qb, pr, attn_pair):
            """Process both heads of qkt pair `pr` together: the two score
            matmuls for one s-tile live in disjoint PE row-groups (lhsT rows
            0-63 vs 64-127) and run concurrently on hardware; one [128,1024]
            ACT exp covers both heads.  Yields once per s-tile so the driver
            can weave filler PE work into the exp-latency bubbles."""
            t0 = qb * TBLK
            ns = 4 * (qb + 1)
            attv = [
                psB.tile([VW, TBLK], F32, tag="attv", name=f"attv{qb}_{pr}_{par}")
                for par in range(2)
            ]
            for si in range(ns):
                diag = si * P >= t0
                ka = si * P - t0 if diag else 0
                stp = ps2.tile([P, 2 * TBLK], F32, tag="st2",
                               name=f"st{qb}_{pr}_{si}")
                for par in range(2):
                    r0 = par * HS
                    nc.tensor.matmul(
                        stp[:, par * TBLK:(par + 1) * TBLK],
                        lhsT=kt_slice(pr, r0, HS, si * P, P),
                        rhs=qt_slice(pr, r0, HS, t0, TBLK),
                        start=True, stop=True,
                    )
                pt = ptp.tile([P, 2 * TBLK], F32R, tag="pt",
                              name=f"pt{qb}_{pr}_{si}")
                if diag:
                    for par in range(2):
                        c0 = par * TBLK + ka
                        nc.scalar.activation(
                            pt[:, c0:(par + 1) * TBLK],
                            stp[:, c0:(par + 1) * TBLK], EXP, scale=0.125)
                        nc.vector.tensor_mul(
                            pt[:, c0:c0 + P], pt[:, c0:c0 + P], tri_sb[:])
                else:
                    nc.scalar.activation(pt[:], stp[:], EXP, scale=0.125)
                for par in range(2):
                    h = 2 * pr + par
                    nc.tensor.matmul(
                        attv[par][:, ka:TBLK],
                        lhsT=v_sb[si][:, h * VW:(h + 1) * VW],
                        rhs=pt[:, par * TBLK + ka:(par + 1) * TBLK],
                        start=(si == 0), stop=(si == ns - 1),
                    )
                yield
            # normalize: recip of denominator row, PE-broadcast, multiply
            for par in range(2):
                r0 = par * HS
                recip = smp.tile([1, TBLK], F32R, tag="recip")
                with nc.allow_low_precision(
                    reason="f32r reciprocal feeds PE broadcast; 19-bit "
                    "mantissa is ample for softmax denominators"
                ):
                    nc.vector.reciprocal(recip[:], attv[par][HS:HS + 1, :])
                bc = psC.tile([HS, TBLK], F32, tag="small",
                              name=f"bc{qb}_{pr}_{par}")
                nc.tensor.matmul(
                    bc[:], lhsT=ones[0:1, 0:HS], rhs=recip[:],
                    start=True, stop=True,
                )
                bcs = smp.tile([HS, TBLK], F32, tag="bcs")
                nc.vector.tensor_copy(bcs[:], bc[:])
                nc.vector.tensor_mul(
                    attn_pair[pr][r0:r0 + HS, :], attv[par][0:HS, :], bcs[:]
                )

        # ---------------- stage-3 piece (one t-tile of one t-block) ---------
        def emit_oproj_tt(qb, lh, tt):
            # lh[hdt] holds att^T rows for global heads (2*hdt, 2*hdt+1)...
            # here indexed so lh[hdt] pairs with w_sb["wo"][hdt]
            t0 = qb * TBLK
            op = psC.tile([P, ES], F32, tag="small", name=f"op{qb}_{tt}")
            nc.tensor.matmul(
                op[:], lhsT=ones[0:1, :], rhs=bias_sb[:],
                start=True, stop=False,
            )
            # pr0 tiles (even hdt) first: they arrive one AllGather earlier
            order = [0, 2, 4, 6, 1, 3, 5, 7]
            for i, hdt in enumerate(order):
                nc.tensor.matmul(
                    op[:],
                    lhsT=lh[hdt][:, tt * P:(tt + 1) * P],
                    rhs=w_sb["wo"][hdt][:],
                    start=False,
                    stop=(i == NCT - 1),
                )
            osb = outp.tile([P, ES], F32, tag="osb", name=f"osb{qb}_{tt}")
            # DVE, not ACT: out-proj fillers run inside exp-bound stretches
            nc.vector.tensor_copy(osb[:], op[:])
            nc.sync.dma_start(out[t0 + tt * P:t0 + (tt + 1) * P, :], osb[:])

        # --------- per-pair AllGather (pr = head pair 0/1 of this core) -----
        # Gathering one head-pair [128, 512] per collective: output rows are
        # rank-major, i.e. block g holds GLOBAL heads (4g+2pr, 4g+2pr+1) =
        # global hd-tile index 2g+pr.  lh list is indexed by wo-row tile.
        def emit_ag(qb, pr, attn_pair, lh):
            ag_in = dramp.tile([P, TBLK], F32R, tag=f"agin{qb}_{pr}")
            nc.sync.dma_start(ag_in[:], attn_pair[pr][:])
            ag_out = dramp.tile([GROUPS * P, TBLK], F32R, tag=f"agout{qb}_{pr}")
            if with_collective:
                nc.gpsimd.collective_compute(
                    "AllGather",
                    mybir.AluOpType.bypass,
                    replica_groups=REPLICA_GROUPS,
                    ins=[ag_in[:].opt()],
                    outs=[ag_out[:].opt()],
                )
            else:  # timing/sim variant: fake the AG with local DMA copies
                for g_ in range(GROUPS):
                    nc.sync.dma_start(
                        ag_out[g_ * P:(g_ + 1) * P, :], ag_in[:])
            for g_ in range(GROUPS):
                t_ = lhp.tile([P, TBLK], F32R, tag="lh",
                              name=f"lh{qb}_{pr}_{g_}")
                nc.sync.dma_start(t_[:], ag_out[g_ * P:(g_ + 1) * P, :])
                lh[2 * g_ + pr] = t_

        # ---------------- emission schedule ----------------
        # stage 1, t-block 0 (DMAs interleaved for fast start)
        xt0 = alloc_xt(0)
        for ci in range(NCT):
            nc.sync.dma_start(w_sb["wq"][ci][:], wq[ci * P:(ci + 1) * P, :])
            nc.sync.dma_start(xt0[ci][:], xT[ci * P:(ci + 1) * P, 0:TBLK])
        for ci in range(NCT):  # wk on HWDGE, wv on SWDGE: parallel sets
            nc.sync.dma_start(w_sb["wk"][ci][:], wk[ci * P:(ci + 1) * P, :])
        for ci in range(NCT):
            nc.sync.dma_start(w_sb["wv"][ci][:], wv[ci * P:(ci + 1) * P, :])
        for chunk in qk_chunks(0) + v_chunks(0):
            chunk()

        def drive_pair(qb, pr, attn_pair, vfill, fillers, stride, off=0):
            """Drive one head pair's s-loop, weaving V fillers (odd units,
            needed by this block's own diagonal s-tiles) and other fillers
            (every `stride` units starting after `off`)."""
            ctr = 0
            for _ in emit_headpair(qb, pr, attn_pair):
                ctr += 1
                if vfill and ctr % 2 == 1:
                    vfill.pop(0)()
                elif (fillers and ctr > off
                      and (ctr - off) % stride == 0):
                    fillers.pop(0)()

        lh_of = {}
        ap_of = {}

        def new_attn_pair(qb):
            ap_of[qb] = [
                attp.tile([P, TBLK], F32R, tag=f"attn{p_}", name=f"at{qb}_{p_}")
                for p_ in range(2)
            ]
            lh_of[qb] = [None] * NCT
            return ap_of[qb]

        def oproj_fillers(qb):
            return [(lambda tt=tt, q=qb: emit_oproj_tt(q, lh_of[q], tt))
                    for tt in range(4)]

        # ---- t-blocks 0 and 1: sequential ----
        for qb in (0, 1):
            emit_x_dma(qb + 1)
            if qb == 0:
                # wo/bias DMAs: needed only from stage 3 on, so they queue
                # behind the t-block-1 x loads
                for ci in range(NCT):
                    nc.sync.dma_start(
                        w_sb["wo"][ci][:], wo[ci * P:(ci + 1) * P, :])
                nc.sync.dma_start(bias_sb[:], bo[:])
            vfill = v_chunks(qb) if qb > 0 else []
            fillers = qk_chunks(qb + 1)
            if qb > 0:
                fillers += oproj_fillers(qb - 1)
            stride = max(2, (8 * (qb + 1)) // max(1, len(fillers)))
            ap = new_attn_pair(qb)
            drive_pair(qb, 0, ap, vfill, fillers, stride)
            emit_ag(qb, 0, ap, lh_of[qb])
            drive_pair(qb, 1, ap, vfill, fillers, stride)
            while vfill:
                vfill.pop(0)()
            while fillers:
                fillers.pop(0)()
            emit_ag(qb, 1, ap, lh_of[qb])

        # ---- t-blocks 2 and 3: interleaved at head-pair granularity ----
        # qb3's s-loops are exp(ACT)-bound while qb2 + the stage-1/3 fillers
        # are PE-rich; alternating their pairs averages the imbalance.
        emit_x_dma(3)
        ap2, ap3 = new_attn_pair(2), new_attn_pair(3)
        oq1 = oproj_fillers(1)
        # (2,0): V(tb2) on odd units; QK(tb3) + 2 oproj(qb1) strided
        drive_pair(2, 0, ap2, v_chunks(2), qk_chunks(3) + oq1[:2], 2)
        emit_ag(2, 0, ap2, lh_of[2])
        # (3,0): V(tb3) on odd units; rest of oproj(qb1) strided
        drive_pair(3, 0, ap3, v_chunks(3), oq1[2:], 6)
        emit_ag(3, 0, ap3, lh_of[3])
        # (2,1): nothing left to fill; exp backlog from (3,0) keeps ACT busy
        drive_pair(2, 1, ap2, [], [], 99)
        emit_ag(2, 1, ap2, lh_of[2])
        # (3,1): oproj(qb2) injected in the second half, once its
        # AllGathered inputs (issued just above) have landed
        oq2 = oproj_fillers(2)
        drive_pair(3, 1, ap3, [], oq2[:2], 3, off=8)
        emit_ag(3, 1, ap3, lh_of[3])
        while oq2:
            oq2.pop(0)()

        # tail: out-projection of the last t-block, two t-tiles per phase:
        # bias + pr0 hd-tiles (landed with the mid-block AllGather) first,
        # so PE has work while the final AllGather is in flight
        lhz = lh_of[NTB - 1]
        tz = (NTB - 1) * TBLK
        for grp in range(2):
            tts = (2 * grp, 2 * grp + 1)
            ops = {}
            for tt in tts:
                op = psC.tile([P, ES], F32, tag="small", name=f"opz{tt}")
                nc.tensor.matmul(
                    op[:], lhsT=ones[0:1, :], rhs=bias_sb[:],
                    start=True, stop=False,
                )
                for hdt in (0, 2, 4, 6):
                    nc.tensor.matmul(
                        op[:],
                        lhsT=lhz[hdt][:, tt * P:(tt + 1) * P],
                        rhs=w_sb["wo"][hdt][:],
                        start=False, stop=False,
                    )
                ops[tt] = op
            for tt in tts:
                for j, hdt in enumerate((1, 3, 5, 7)):
                    nc.tensor.matmul(
                        ops[tt][:],
                        lhsT=lhz[hdt][:, tt * P:(tt + 1) * P],
                        rhs=w_sb["wo"][hdt][:],
                        start=False, stop=(j == 3),
                    )
                osb = outp.tile([P, ES], F32, tag="osb", name=f"osbz{tt}")
                nc.vector.tensor_copy(osb[:], ops[tt][:])
                nc.sync.dma_start(
                    out[tz + tt * P:tz + (tt + 1) * P, :], osb[:])

    nc.compile()
    return nc


_NC_CACHE = {}


def _get_nc(with_collective=True):
    key = with_collective
    if key not in _NC_CACHE:
        _NC_CACHE[key] = build_nc(with_collective)
    return _NC_CACHE[key]


def make_in_maps(x, Wq, Wk, Wv, Wo, bo):
    tri = np.ascontiguousarray(np.triu(np.ones((P, P), dtype=np.float32)))
    onesc = np.ones((1, P), dtype=np.float32)
    vones = np.ones((P, HPG), dtype=np.float32)
    in_maps = []
    for c in range(N_CORES):
        b, g = c // GROUPS, c % GROUPS
        hs_ = slice(g * HPG, (g + 1) * HPG)
        in_maps.append({
            "xT": np.ascontiguousarray(x[b].T),
            "wq": np.ascontiguousarray(
                Wq[hs_].transpose(1, 0, 2).reshape(C, HD)),
            "wk": np.ascontiguousarray(
                Wk[hs_].transpose(1, 0, 2).reshape(C, HD)),
            "wv": np.ascontiguousarray(
                Wv[hs_].transpose(1, 0, 2).reshape(C, HD)),
            "wo": np.ascontiguousarray(Wo[:, g * ES:(g + 1) * ES]),
            "bo": np.ascontiguousarray(bo[g * ES:(g + 1) * ES].reshape(1, ES)),
            "tri": tri,
            "onesc": onesc,
            "vones": vones,
        })
    return in_maps


def kernel(x, Wq, Wk, Wv, Wo, bo):
    x = np.asarray(x, dtype=np.float32)
    Wq = np.asarray(Wq, dtype=np.float32)
    Wk = np.asarray(Wk, dtype=np.float32)
    Wv = np.asarray(Wv, dtype=np.float32)
    Wo = np.asarray(Wo, dtype=np.float32)
    bo = np.asarray(bo, dtype=np.float32)

    nc = _get_nc(with_collective=True)
    in_maps = make_in_maps(x, Wq, Wk, Wv, Wo, bo)
    res = run_bass_kernel_spmd(nc, in_maps, core_ids=list(range(N_CORES)))

    out = np.empty((B, T, E), dtype=np.float32)
    for c in range(N_CORES):
        b, g = c // GROUPS, c % GROUPS
        out[b, :, g * ES:(g + 1) * ES] = res.results[c]["out"]
    return out



# revision 2
# speedup vs baseline: 1.2225x; 1.2225x over previous
"""Multi-head causal self-attention on 8 TRN2 NeuronCores — v2 (bf16).

Problem (nn_MultiHeadAttention): B=2, T=2048, C=1024, H=16 heads, hs=64.

Sharding: core c -> (batch b = c//4, head-group g = c%4, 4 heads each).
AllGather of normalized attention outputs across the 4 cores of a batch
(replica groups [0-3], [4-7]); each core computes a disjoint 256-column
slice of the output projection, TRANSPOSED (out^T [256, 2048]); host
transposes + concatenates.

v2 design (cost-model-driven):
- All matmul I/O in bf16 (1 cycle/row at any size; halves SBUF/DMA bytes).
  PSUM accumulation stays f32; rel-err budget 2e-2 >> bf16 noise.
- Scores S^T [s,q] per head pair as before, but causal mask applied ON PE:
  an extra accumulate matmul (lhsT=I, rhs=tri(-32768)) onto the diagonal
  128-block, so exp needs no DVE mask multiply.
- AV in [q,d] layout: att[q,h] accumulated per 128-q-subtile with
  lhsT = pt-slice (cost 64/matmul instead of 512). Softmax denominator via
  N=1 matmuls (rhs=ones) into a separate PSUM bank. Normalization becomes a
  reciprocal [128,8] + stride-0-broadcast multiplies (cheap DVE ops).
- att^T for the collective produced by XBAR DMA transpose (no PE/PSUM).
- Output projection computes out^T [e,t] (lhsT=Wo tile, rhs=AllGathered
  att^T), so the bias is per-partition and folds into the PSUM->SBUF copy.
- PSUM bank discipline: matmul start=True zeroes a whole 2KB bank, so each
  bank gets exactly one start and slice-groups accumulate with start=False.
  Banks: scores ping-pong 2x[128,1024] (4), att accum 2x[128,512] (2),
  denominators [128,512] (1), shared small pool (1) = 8.
- DMAs batched (one per weight matrix, per x row-tile, per lh gather) to
  keep the serial HWDGE setup (~630ns each) off the critical path.
"""

import numpy as np
import ml_dtypes
from contextlib import ExitStack

import concourse.bass as bass
import concourse.mybir as mybir
import concourse.tile as tile
from concourse import bacc
from concourse.bass_utils import run_bass_kernel_spmd

F32 = mybir.dt.float32
BF16 = mybir.dt.bfloat16
BF = ml_dtypes.bfloat16
EXP = mybir.ActivationFunctionType.Exp
MULT = mybir.AluOpType.mult

N_CORES = 8
B = 2
T = 2048
C = 1024
NH = 16
HS = 64
E = 1024
GROUPS = 4
HPG = NH // GROUPS   # 4 heads per core
ES = E // GROUPS     # 256 output columns per core
HD = HPG * HS        # 256 local attention-output rows

P = 128
TBLK = 512
NTB = T // TBLK      # 4
NCT = C // P         # 8
NST = T // P         # 16

REPLICA_GROUPS = [[0, 1, 2, 3], [4, 5, 6, 7]]


def build_nc(with_collective=True):
    nc = bacc.Bacc(
        "TRN2", target_bir_lowering=False, debug=False, num_devices=N_CORES
    )

    xT = nc.dram_tensor("xT", [C, T], BF16, kind="ExternalInput").ap()
    wq = nc.dram_tensor("wq", [C, HD], BF16, kind="ExternalInput").ap()
    wk = nc.dram_tensor("wk", [C, HD], BF16, kind="ExternalInput").ap()
    wv = nc.dram_tensor("wv", [C, HD], BF16, kind="ExternalInput").ap()
    wo = nc.dram_tensor("wo", [E, ES], BF16, kind="ExternalInput").ap()
    bo2 = nc.dram_tensor("bo2", [P, 2], F32, kind="ExternalInput").ap()
    trineg = nc.dram_tensor("trineg", [P, P], BF16, kind="ExternalInput").ap()
    ident = nc.dram_tensor("ident", [P, P], BF16, kind="ExternalInput").ap()
    onescol = nc.dram_tensor("onescol", [P, 1], BF16, kind="ExternalInput").ap()
    outT = nc.dram_tensor("outT", [ES, T], F32, kind="ExternalOutput").ap()

    with tile.TileContext(nc) as tc, ExitStack() as ctx:
        wp = ctx.enter_context(tc.tile_pool(name="wp", bufs=1))
        xp = ctx.enter_context(tc.tile_pool(name="xp", bufs=1))
        qkp = ctx.enter_context(tc.tile_pool(name="qkp", bufs=1))
        vp = ctx.enter_context(tc.tile_pool(name="vp", bufs=1))
        ptp = ctx.enter_context(tc.tile_pool(name="ptp", bufs=8))
        asp = ctx.enter_context(tc.tile_pool(name="asp", bufs=8))
        atp = ctx.enter_context(tc.tile_pool(name="atp", bufs=4))
        lhp = ctx.enter_context(tc.tile_pool(name="lhp", bufs=6))
        rp = ctx.enter_context(tc.tile_pool(name="rp", bufs=8))
        osp = ctx.enter_context(tc.tile_pool(name="osp", bufs=4))
        # PSUM: 8 banks. ps2 = scores ping-pong (2x2 banks), psA = att
        # accumulators (2x1), psD = denominators (1), psC = small shared (1).
        ps2 = ctx.enter_context(tc.tile_pool(name="ps2", bufs=2, space="PSUM"))
        psA = ctx.enter_context(tc.tile_pool(name="psA", bufs=1, space="PSUM"))
        psD = ctx.enter_context(tc.tile_pool(name="psD", bufs=1, space="PSUM"))
        psC = ctx.enter_context(tc.tile_pool(name="psC", bufs=1, space="PSUM"))
        dramp = ctx.enter_context(tc.tile_pool(name="dramp", bufs=1, space="DRAM"))

        # ---- constants (tiles; DMAs issued after the hot-path loads) ----
        tri_sb = wp.tile([P, P], BF16, tag="tri")
        id_sb = wp.tile([P, P], BF16, tag="id")
        ones_sb = wp.tile([P, 1], BF16, tag="ones")
        bo_sb = wp.tile([P, 2], F32, tag="bo")

        def load_consts():
            nc.sync.dma_start(tri_sb[:], trineg[:])
            nc.sync.dma_start(id_sb[:], ident[:])
            nc.sync.dma_start(ones_sb[:], onescol[:])

        # ---- big weight tiles, one DMA each: [128, 8*width] ----
        w_sb = {}

        def load_w(name, dram, width):
            t_ = wp.tile([P, NCT * width], BF16, tag=name)
            nc.sync.dma_start(
                t_[:].rearrange("p (c e) -> p c e", c=NCT),
                dram.rearrange("(c p) e -> p c e", p=P),
            )
            w_sb[name] = t_

        # ---- x row-tiles: tb0 chunk first (unblocks stage-1 fast), rest
        # loaded in a second wave ----
        x_sb = []

        def load_x_tb0(ci):
            t_ = xp.tile([P, T], BF16, tag=f"x{ci}", name=f"x{ci}")
            nc.sync.dma_start(t_[:, 0:TBLK], xT[ci * P:(ci + 1) * P, 0:TBLK])
            x_sb.append(t_)

        def load_x_rest(ci):
            nc.sync.dma_start(
                x_sb[ci][:, TBLK:T], xT[ci * P:(ci + 1) * P, TBLK:T])

        # merged Q^T/K^T per head pair: col = tb*1024 + which*512 + t_local
        qkt = [qkp.tile([P, 2 * T], BF16, tag=f"qk{p_}", name=f"qk{p_}")
               for p_ in range(2)]

        def qt_slice(pr, hh, t0, tn):
            tb, tl = t0 // TBLK, t0 % TBLK
            base = tb * 1024 + tl
            return qkt[pr][hh * HS:(hh + 1) * HS, base:base + tn]

        def kt_slice(pr, hh, s0, sn):
            tb, sl = s0 // TBLK, s0 % TBLK
            base = tb * 1024 + TBLK + sl
            return qkt[pr][hh * HS:(hh + 1) * HS, base:base + sn]

        v_sb = [vp.tile([P, HD], BF16, tag=f"v{st}", name=f"v{st}")
                for st in range(NST)]

        # ---------------- stage-1 pieces ----------------
        def emit_qk_proj(tb, pr, which, pool=None, ptag="small"):
            pool = pool or psC
            ps_ = pool.tile([P, TBLK], F32, tag=ptag,
                            name=f"qkps{tb}_{pr}_{which}")
            wn = "wq" if which == 0 else "wk"
            for ci in range(NCT):
                nc.tensor.matmul(
                    ps_[:],
                    lhsT=w_sb[wn][:, ci * HD + pr * P:ci * HD + (pr + 1) * P],
                    rhs=x_sb[ci][:, tb * TBLK:(tb + 1) * TBLK],
                    start=(ci == 0), stop=(ci == NCT - 1),
                )
            base = tb * 1024 + which * TBLK
            nc.vector.tensor_copy(qkt[pr][:, base:base + TBLK], ps_[:])

        def emit_v_proj(st, pool=None, ptag="small"):
            pool = pool or psC
            ps_ = pool.tile([P, TBLK], F32, tag=ptag, name=f"vps{st}")
            for ci in range(NCT):
                nc.tensor.matmul(
                    ps_[:, 0:HD],
                    lhsT=x_sb[ci][:, st * P:(st + 1) * P],
                    rhs=w_sb["wv"][:, ci * HD:(ci + 1) * HD],
                    start=(ci == 0), stop=(ci == NCT - 1),
                )
            nc.vector.tensor_copy(v_sb[st][:], ps_[:, 0:HD])

        def qk_chunks(tb):
            return [lambda tb=tb, pr=pr, w=w: emit_qk_proj(tb, pr, w)
                    for pr in range(2) for w in range(2)]

        def v_chunks(tb):
            return [lambda st=st: emit_v_proj(st)
                    for st in range(4 * tb, 4 * tb + 4)]

        # ------- stage-2: s-loop of one head pair of one t-block ------
        att_of = {}   # qb -> [A01, A23] psum tiles [128, 512]
        den_of = {}   # qb -> psum tile [128, 512] (cols pr*8 + sub*2 + hh)

        def new_qb_psum(qb):
            att_of[qb] = [
                psA.tile([P, 4 * P], F32, tag=f"att{b_}", name=f"att{qb}_{b_}")
                for b_ in range(2)
            ]
            den_of[qb] = psD.tile([P, TBLK], F32, tag="den", name=f"den{qb}")

        def emit_headpair(qb, pr):
            """Scores + exp + AV/denominator accumulation for heads
            (2pr, 2pr+1). Yields once per s-tile for the filler driver."""
            t0 = qb * TBLK
            ns = 4 * (qb + 1)
            att, den = att_of[qb], den_of[qb]
            for si in range(ns):
                diag = si * P >= t0
                ka = si * P - t0 if diag else 0
                stp = ps2.tile([P, 2 * TBLK], F32, tag="st2",
                               name=f"st{qb}_{pr}_{si}")
                for hh in range(2):
                    c0 = hh * TBLK + ka
                    nc.tensor.matmul(
                        stp[:, c0:(hh + 1) * TBLK],
                        lhsT=kt_slice(pr, hh, si * P, P),
                        rhs=qt_slice(pr, hh, t0 + ka, TBLK - ka),
                        start=True, stop=not diag,
                    )
                    if diag:
                        nc.tensor.matmul(
                            stp[:, c0:c0 + P], lhsT=id_sb[:], rhs=tri_sb[:],
                            start=False, stop=True, skip_group_check=True,
                        )
                pt = ptp.tile([P, 2 * TBLK], BF16, tag="pt",
                              name=f"pt{qb}_{pr}_{si}")
                if diag:
                    for hh in range(2):
                        c0 = hh * TBLK + ka
                        nc.scalar.activation(
                            pt[:, c0:(hh + 1) * TBLK],
                            stp[:, c0:(hh + 1) * TBLK], EXP, scale=0.125)
                else:
                    nc.scalar.activation(pt[:], stp[:], EXP, scale=0.125)
                sub0 = ka // P
                for sub in range(sub0, 4):
                    last = si == 4 * qb + sub
                    for hh in range(2):
                        h = 2 * pr + hh
                        first = pr == 0 and hh == 0 and si == 0 and sub % 2 == 0
                        pslice = pt[:, hh * TBLK + sub * P:
                                    hh * TBLK + (sub + 1) * P]
                        nc.tensor.matmul(
                            att[sub // 2][:, (sub % 2) * HD + h * HS:
                                          (sub % 2) * HD + (h + 1) * HS],
                            lhsT=pslice, rhs=v_sb[si][:, h * HS:(h + 1) * HS],
                            start=first, stop=last, skip_group_check=True,
                        )
                        dfirst = pr == 0 and hh == 0 and si == 0 and sub == 0
                        dcol = pr * 8 + sub * 2 + hh
                        nc.tensor.matmul(
                            den[:, dcol:dcol + 1],
                            lhsT=pslice, rhs=ones_sb[:],
                            start=dfirst, stop=last, skip_group_check=True,
                        )
                # normalize q-subtiles whose denominators just completed
                # (all but the last, which emit_finish_pair handles)
                if si >= 4 * qb and si - 4 * qb < 3:
                    emit_norm_sub(qb, pr, si - 4 * qb)
                yield

        # ------- normalize (per-sub, early) + AllGather for one pair -------
        # att stays in [q, d] layout end-to-end on this side; the transpose
        # to [d, t] happens in the lh load from DRAM via the XBAR.
        lh_of = {}
        asb_of = {}

        def emit_norm_sub(qb, pr, sub):
            """Reciprocal + normalize one q-subtile as soon as its
            denominators are complete (after s-tile si = 4qb+sub)."""
            att, den = att_of[qb], den_of[qb]
            if sub == 0:
                asb_of[(qb, pr)] = asp.tile(
                    [P, 4 * P], BF16, tag=f"asb{pr}", name=f"asb{qb}_{pr}")
            rec = rp.tile([P, 2], F32, tag="rec", name=f"rec{qb}_{pr}_{sub}")
            dcol = pr * 8 + sub * 2
            nc.vector.reciprocal(rec[:], den[:, dcol:dcol + 2])
            in0 = att[sub // 2][:, (sub % 2) * HD + pr * P:
                                (sub % 2) * HD + (pr + 1) * P]
            nc.vector.tensor_tensor(
                asb_of[(qb, pr)][:, sub * P:(sub + 1) * P]
                .rearrange("p (h d) -> p h d", h=2),
                in0.rearrange("p (h d) -> p h d", h=2),
                rec[:].broadcast_to([P, 2, HS]),
                MULT,
            )

        def emit_finish_pair(qb, pr):
            emit_norm_sub(qb, pr, 3)
            a_sb = asb_of[(qb, pr)]
            # AllGather this pair's att [512 t, 128 d] (t-major rows)
            ag_in = dramp.tile([4 * P, P], BF16, tag=f"agin{qb}_{pr}")
            nc.sync.dma_start(
                ag_in[:].rearrange("(s q) d -> q s d", q=P),
                a_sb[:].rearrange("q (s d) -> q s d", s=4),
            )
            ag_out = dramp.tile([GROUPS * 4 * P, P], BF16,
                                tag=f"agout{qb}_{pr}")
            if with_collective:
                nc.gpsimd.collective_compute(
                    "AllGather",
                    mybir.AluOpType.bypass,
                    replica_groups=REPLICA_GROUPS,
                    ins=[ag_in[:].opt()],
                    outs=[ag_out[:].opt()],
                )
            else:  # timing/sim variant: fake the AG with local DMA copies
                nc.sync.dma_start(
                    ag_out[:].rearrange("(g t) d -> t g d", t=4 * P),
                    ag_in[:].rearrange("(g t) d -> t g d", g=1)
                    .broadcast_to([4 * P, GROUPS, P]),
                )
            # transpose-load the gathered chunks: lh[g] = [128 d, 512 t]
            lhs = []
            for g in range(GROUPS):
                lh_t = lhp.tile([P, TBLK], BF16, tag=f"lh{pr}_{g}",
                                name=f"lh{qb}_{pr}_{g}")
                nc.sync.dma_start_transpose(
                    lh_t[:], ag_out[g * 4 * P:(g + 1) * 4 * P, :])
                lhs.append(lh_t)
            lh_of[(qb, pr)] = lhs

        # ---------------- stage-3: out^T projection ----------------
        def emit_oproj(qb, et, pool=None, tag="small",
                       korder=(0, 2, 4, 6, 1, 3, 5, 7), split_out=False):
            pool = pool or psC
            op = pool.tile([P, TBLK], F32, tag=tag, name=f"op{qb}_{et}")
            for j, k in enumerate(korder):
                nc.tensor.matmul(
                    op[:],
                    lhsT=w_sb["wo"][:, k * ES + et * P:k * ES + (et + 1) * P],
                    rhs=lh_of[(qb, k % 2)][k // 2][:],
                    start=(j == 0), stop=(j == NCT - 1),
                )
            o_sb = osp.tile([P, TBLK], F32, tag="osb", name=f"osb{qb}_{et}")
            nh = 2 if split_out else 1
            for h in range(nh):
                sl = slice(h * TBLK // nh, (h + 1) * TBLK // nh)
                nc.vector.tensor_scalar_add(
                    o_sb[:, sl], op[:, sl], bo_sb[:, et:et + 1])
                nc.sync.dma_start(
                    outT[et * P:(et + 1) * P,
                         qb * TBLK + sl.start:qb * TBLK + sl.stop],
                    o_sb[:, sl])

        def oproj_fillers(qb):
            return [lambda et=et, q=qb: emit_oproj(q, et) for et in range(2)]

        # ---------------- emission schedule ----------------
        load_w("wq", wq, HD)
        for ci in range(NCT):
            load_x_tb0(ci)
        load_w("wk", wk, HD)
        load_w("wv", wv, HD)
        load_consts()
        # startup chunks spread across the (still free) PSUM banks so they
        # don't serialize on the single shared bank
        emit_qk_proj(0, 0, 0, psC, "small")
        emit_qk_proj(0, 0, 1, psD, "den")
        emit_v_proj(0, psA, "att0")
        emit_v_proj(1, psA, "att1")
        emit_v_proj(2, psC, "small")
        emit_v_proj(3, psD, "den")
        for ci in range(NCT):
            load_x_rest(ci)
        load_w("wo", wo, ES)
        nc.sync.dma_start(bo_sb[:], bo2[:])

        def drive_pair(qb, pr, vfill, fillers, stride, off=0):
            ctr = 0
            for _ in emit_headpair(qb, pr):
                ctr += 1
                if vfill and ctr % 2 == 1:
                    vfill.pop(0)()
                elif (fillers and ctr > off
                      and (ctr - off) % stride == 0):
                    fillers.pop(0)()

        # fillers per qb: v for qb's own diagonal on odd units (vfill),
        # stage-1 for qb+1 and delayed out-projections strided (fillers)
        qkp1 = [lambda w=w: emit_qk_proj(0, 1, w) for w in range(2)]
        plan = {
            0: ([], qkp1 + qk_chunks(1)),
            1: (v_chunks(1), qk_chunks(2)),
            2: (v_chunks(2), qk_chunks(3)),
            3: (v_chunks(3), oproj_fillers(0) + oproj_fillers(1)),
        }
        for qb in range(NTB):
            new_qb_psum(qb)
            vfill, fillers = plan[qb]
            vfill, fillers = list(vfill), list(fillers)
            ns = 4 * (qb + 1)
            stride = max(1, (2 * ns) // max(1, len(fillers) + 1))
            drive_pair(qb, 0, vfill, fillers, stride)
            emit_finish_pair(qb, 0)
            drive_pair(qb, 1, vfill, fillers, stride)
            while vfill:
                vfill.pop(0)()
            while fillers:
                fillers.pop(0)()
            emit_finish_pair(qb, 1)

        # tail: out-projections of qb2 (its lh landed long ago — keeps PE
        # busy and hot while qb3-pair1's AllGather chain is in flight),
        # then qb3's. The retired den bank doubles as a second accumulator
        # so the two e-tiles run in parallel instead of serializing on psC.
        emit_oproj(2, 0, psC, "small")
        emit_oproj(2, 1, psD, "den")
        late = (1, 3, 5, 7, 0, 2, 4, 6)
        emit_oproj(3, 0, psC, "small", korder=late, split_out=True)
        emit_oproj(3, 1, psD, "den", korder=late, split_out=True)

    nc.compile()
    return nc


_NC_CACHE = {}


def _get_nc(with_collective=True):
    key = with_collective
    if key not in _NC_CACHE:
        _NC_CACHE[key] = build_nc(with_collective)
    return _NC_CACHE[key]


def make_in_maps(x, Wq, Wk, Wv, Wo, bo):
    tri_neg = np.ascontiguousarray(
        np.tril(np.full((P, P), -32768.0, dtype=np.float32), -1)).astype(BF)
    ident = np.eye(P, dtype=np.float32).astype(BF)
    onescol = np.ones((P, 1), dtype=BF)
    in_maps = []
    for c in range(N_CORES):
        b, g = c // GROUPS, c % GROUPS
        hs_ = slice(g * HPG, (g + 1) * HPG)
        bo_sl = bo[g * ES:(g + 1) * ES].astype(np.float32)
        in_maps.append({
            "xT": np.ascontiguousarray(x[b].T).astype(BF),
            "wq": np.ascontiguousarray(
                Wq[hs_].transpose(1, 0, 2).reshape(C, HD)).astype(BF),
            "wk": np.ascontiguousarray(
                Wk[hs_].transpose(1, 0, 2).reshape(C, HD)).astype(BF),
            "wv": np.ascontiguousarray(
                Wv[hs_].transpose(1, 0, 2).reshape(C, HD)).astype(BF),
            "wo": np.ascontiguousarray(Wo[:, g * ES:(g + 1) * ES]).astype(BF),
            "bo2": np.ascontiguousarray(bo_sl.reshape(2, P).T),
            "trineg": tri_neg,
            "ident": ident,
            "onescol": onescol,
        })
    return in_maps


def kernel(x, Wq, Wk, Wv, Wo, bo):
    x = np.asarray(x, dtype=np.float32)
    Wq = np.asarray(Wq, dtype=np.float32)
    Wk = np.asarray(Wk, dtype=np.float32)
    Wv = np.asarray(Wv, dtype=np.float32)
    Wo = np.asarray(Wo, dtype=np.float32)
    bo = np.asarray(bo, dtype=np.float32)

    nc = _get_nc(with_collective=True)
    in_maps = make_in_maps(x, Wq, Wk, Wv, Wo, bo)
    res = run_bass_kernel_spmd(nc, in_maps, core_ids=list(range(N_CORES)))

    out = np.empty((B, T, E), dtype=np.float32)
    for c in range(N_CORES):
        b, g = c // GROUPS, c % GROUPS
        out[b, :, g * ES:(g + 1) * ES] = res.results[c]["outT"].T
    return out


# revision 4
# speedup vs baseline: 1.2622x; 1.0325x over previous
"""Multi-head causal self-attention on 8 TRN2 NeuronCores — v2 (bf16).

Problem (nn_MultiHeadAttention): B=2, T=2048, C=1024, H=16 heads, hs=64.

Sharding: core c -> (batch b = c//4, head-group g = c%4, 4 heads each).
AllGather of normalized attention outputs across the 4 cores of a batch
(replica groups [0-3], [4-7]); each core computes a disjoint 256-column
slice of the output projection, TRANSPOSED (out^T [256, 2048]); host
transposes + concatenates.

v2 design (cost-model-driven; matmul cost = out free-size x cycles/row):
- All matmul I/O in bf16 (1 cycle/row at any size; halves SBUF/DMA bytes).
  PSUM accumulation stays f32; rel-err budget 2e-2 >> bf16 noise
  (measured rel err ~3.5e-3).
- Scores S^T [s,q] per head pair, rhs q-range trimmed to the causal part;
  the diagonal 128-block is masked by a DVE triu-multiply on the exp'd
  probabilities (bf16, 2x DVE mode).
- AV in [q,d] layout: att[q,h] accumulated per 128-q-subtile with
  lhsT = pt-slice (cost 64+1/matmul instead of 512). Softmax denominators
  via N=1 matmuls (rhs=ones) into a separate PSUM bank. Normalization is a
  [128,2] reciprocal + stride-0-broadcast multiply per subtile, emitted
  EARLY (as soon as each subtile's denominators complete mid s-loop).
- The pair's att [512 t, 128 d] is AllGathered t-major; the transpose to
  [d, t] for the output projection happens for free in the gather readback
  via XBAR DMA-transpose loads (no PE/PSUM/DVE involvement).
- Output projection computes out^T [e,t] (lhsT=Wo tile, rhs=AllGathered
  att^T), so the bias is per-partition and folds into the PSUM->SBUF copy.
- PSUM bank discipline: matmul start=True zeroes a whole 2KB bank, so each
  bank gets exactly one start and slice-groups accumulate with start=False.
  Banks: scores ping-pong 2x[128,1024] (4), att accum 2x[128,512] (2),
  denominators [128,512] (1), shared small pool (1) = 8. GPSIMD cannot
  touch PSUM, so all PSUM evacuation rides on DVE.
- DMAs batched (one per weight matrix, per x row-tile, per lh gather) to
  keep the serial HWDGE setup (~630ns each) off the critical path; x loads
  split tb0-first so stage-1 starts early.
- Tail: the final out-projections order their lh operands latest-first so
  the tile scheduler cannot hoist them ahead of the last s-loop (PE
  head-of-line), warm-up matmuls keep the PE p-state at full clock across
  the final AllGather latency, and the last stores are split in halves to
  pipeline copy/DMA. qb2's out-projection is deferred to the same window.
"""

import numpy as np
import ml_dtypes
from contextlib import ExitStack

import concourse.bass as bass
import concourse.mybir as mybir
import concourse.tile as tile
from concourse import bacc
from concourse.bass_utils import run_bass_kernel_spmd

F32 = mybir.dt.float32
BF16 = mybir.dt.bfloat16
BF = ml_dtypes.bfloat16
EXP = mybir.ActivationFunctionType.Exp
MULT = mybir.AluOpType.mult

N_CORES = 8
B = 2
T = 2048
C = 1024
NH = 16
HS = 64
E = 1024
GROUPS = 4
HPG = NH // GROUPS   # 4 heads per core
ES = E // GROUPS     # 256 output columns per core
HD = HPG * HS        # 256 local attention-output rows

P = 128
TBLK = 512
NTB = T // TBLK      # 4
NCT = C // P         # 8
NST = T // P         # 16

REPLICA_GROUPS = [[0, 1, 2, 3], [4, 5, 6, 7]]


def build_nc(with_collective=True):
    nc = bacc.Bacc(
        "TRN2", target_bir_lowering=False, debug=False, num_devices=N_CORES
    )

    xT = nc.dram_tensor("xT", [C, T], BF16, kind="ExternalInput").ap()
    wq = nc.dram_tensor("wq", [C, HD], BF16, kind="ExternalInput").ap()
    wk = nc.dram_tensor("wk", [C, HD], BF16, kind="ExternalInput").ap()
    wv = nc.dram_tensor("wv", [C, HD], BF16, kind="ExternalInput").ap()
    wo = nc.dram_tensor("wo", [E, ES], BF16, kind="ExternalInput").ap()
    bo2 = nc.dram_tensor("bo2", [P, 2], F32, kind="ExternalInput").ap()
    trineg = nc.dram_tensor("trineg", [P, P], BF16, kind="ExternalInput").ap()
    onescol = nc.dram_tensor("onescol", [P, 1], BF16, kind="ExternalInput").ap()
    outT = nc.dram_tensor("outT", [ES, T], F32, kind="ExternalOutput").ap()

    with tile.TileContext(nc) as tc, ExitStack() as ctx:
        wp = ctx.enter_context(tc.tile_pool(name="wp", bufs=1))
        xp = ctx.enter_context(tc.tile_pool(name="xp", bufs=1))
        qkp = ctx.enter_context(tc.tile_pool(name="qkp", bufs=1))
        vp = ctx.enter_context(tc.tile_pool(name="vp", bufs=1))
        ptp = ctx.enter_context(tc.tile_pool(name="ptp", bufs=8))
        asp = ctx.enter_context(tc.tile_pool(name="asp", bufs=8))
        atp = ctx.enter_context(tc.tile_pool(name="atp", bufs=4))
        lhp = ctx.enter_context(tc.tile_pool(name="lhp", bufs=6))
        rp = ctx.enter_context(tc.tile_pool(name="rp", bufs=8))
        osp = ctx.enter_context(tc.tile_pool(name="osp", bufs=4))
        # PSUM: 8 banks. ps2 = scores ping-pong (2x2 banks), psA = att
        # accumulators (2x1), psD = denominators (1), psC = small shared (1).
        ps2 = ctx.enter_context(tc.tile_pool(name="ps2", bufs=2, space="PSUM"))
        psA = ctx.enter_context(tc.tile_pool(name="psA", bufs=1, space="PSUM"))
        psD = ctx.enter_context(tc.tile_pool(name="psD", bufs=1, space="PSUM"))
        psC = ctx.enter_context(tc.tile_pool(name="psC", bufs=1, space="PSUM"))
        dramp = ctx.enter_context(tc.tile_pool(name="dramp", bufs=1, space="DRAM"))

        # ---- constants (tiles; DMAs issued after the hot-path loads) ----
        tri_sb = wp.tile([P, P], BF16, tag="tri")
        ones_sb = wp.tile([P, 1], BF16, tag="ones")
        bo_sb = wp.tile([P, 2], F32, tag="bo")

        def load_consts():
            nc.sync.dma_start(tri_sb[:], trineg[:])
            nc.sync.dma_start(ones_sb[:], onescol[:])

        # ---- big weight tiles, one DMA each: [128, 8*width] ----
        w_sb = {}

        def load_w(name, dram, width):
            t_ = wp.tile([P, NCT * width], BF16, tag=name)
            nc.sync.dma_start(
                t_[:].rearrange("p (c e) -> p c e", c=NCT),
                dram.rearrange("(c p) e -> p c e", p=P),
            )
            w_sb[name] = t_

        # ---- x row-tiles: tb0 chunk first (unblocks stage-1 fast), rest
        # loaded in a second wave ----
        x_sb = []

        def load_x_tb0(ci):
            t_ = xp.tile([P, T], BF16, tag=f"x{ci}", name=f"x{ci}")
            nc.sync.dma_start(t_[:, 0:TBLK], xT[ci * P:(ci + 1) * P, 0:TBLK])
            x_sb.append(t_)

        def load_x_rest(ci):
            nc.sync.dma_start(
                x_sb[ci][:, TBLK:T], xT[ci * P:(ci + 1) * P, TBLK:T])

        # merged Q^T/K^T per head pair: col = tb*1024 + which*512 + t_local
        qkt = [qkp.tile([P, 2 * T], BF16, tag=f"qk{p_}", name=f"qk{p_}")
               for p_ in range(2)]

        def qt_slice(pr, hh, t0, tn):
            tb, tl = t0 // TBLK, t0 % TBLK
            base = tb * 1024 + tl
            return qkt[pr][hh * HS:(hh + 1) * HS, base:base + tn]

        def kt_slice(pr, hh, s0, sn):
            tb, sl = s0 // TBLK, s0 % TBLK
            base = tb * 1024 + TBLK + sl
            return qkt[pr][hh * HS:(hh + 1) * HS, base:base + sn]

        v_sb = [vp.tile([P, HD], BF16, tag=f"v{st}", name=f"v{st}")
                for st in range(NST)]

        # ---------------- stage-1 pieces ----------------
        def emit_qk_proj(tb, pr, which, pool=None, ptag="small"):
            pool = pool or psC
            ps_ = pool.tile([P, TBLK], F32, tag=ptag,
                            name=f"qkps{tb}_{pr}_{which}")
            wn = "wq" if which == 0 else "wk"
            for ci in range(NCT):
                nc.tensor.matmul(
                    ps_[:],
                    lhsT=w_sb[wn][:, ci * HD + pr * P:ci * HD + (pr + 1) * P],
                    rhs=x_sb[ci][:, tb * TBLK:(tb + 1) * TBLK],
                    start=(ci == 0), stop=(ci == NCT - 1),
                )
            base = tb * 1024 + which * TBLK
            nc.vector.tensor_copy(qkt[pr][:, base:base + TBLK], ps_[:])

        def emit_v_proj(st, pool=None, ptag="small"):
            pool = pool or psC
            ps_ = pool.tile([P, TBLK], F32, tag=ptag, name=f"vps{st}")
            for ci in range(NCT):
                nc.tensor.matmul(
                    ps_[:, 0:HD],
                    lhsT=x_sb[ci][:, st * P:(st + 1) * P],
                    rhs=w_sb["wv"][:, ci * HD:(ci + 1) * HD],
                    start=(ci == 0), stop=(ci == NCT - 1),
                )
            nc.vector.tensor_copy(v_sb[st][:], ps_[:, 0:HD])

        def qk_chunks(tb):
            return [lambda tb=tb, pr=pr, w=w: emit_qk_proj(tb, pr, w)
                    for pr in range(2) for w in range(2)]

        def v_chunks(tb):
            return [lambda st=st: emit_v_proj(st)
                    for st in range(4 * tb, 4 * tb + 4)]

        # ------- stage-2: s-loop of one head pair of one t-block ------
        att_of = {}   # qb -> [A01, A23] psum tiles [128, 512]
        den_of = {}   # qb -> psum tile [128, 512] (cols pr*8 + sub*2 + hh)

        def new_qb_psum(qb):
            att_of[qb] = [
                psA.tile([P, 4 * P], F32, tag=f"att{b_}", name=f"att{qb}_{b_}")
                for b_ in range(2)
            ]
            den_of[qb] = psD.tile([P, TBLK], F32, tag="den", name=f"den{qb}")

        def emit_headpair(qb, pr):
            """Scores + exp + AV/denominator accumulation for heads
            (2pr, 2pr+1). Yields once per s-tile for the filler driver."""
            t0 = qb * TBLK
            ns = 4 * (qb + 1)
            att, den = att_of[qb], den_of[qb]
            for si in range(ns):
                diag = si * P >= t0
                ka = si * P - t0 if diag else 0
                stp = ps2.tile([P, 2 * TBLK], F32, tag="st2",
                               name=f"st{qb}_{pr}_{si}")
                for hh in range(2):
                    c0 = hh * TBLK + ka
                    nc.tensor.matmul(
                        stp[:, c0:(hh + 1) * TBLK],
                        lhsT=kt_slice(pr, hh, si * P, P),
                        rhs=qt_slice(pr, hh, t0 + ka, TBLK - ka),
                        start=True, stop=True,
                    )
                pt = ptp.tile([P, 2 * TBLK], BF16, tag="pt",
                              name=f"pt{qb}_{pr}_{si}")
                if diag:
                    for hh in range(2):
                        c0 = hh * TBLK + ka
                        nc.scalar.activation(
                            pt[:, c0:(hh + 1) * TBLK],
                            stp[:, c0:(hh + 1) * TBLK], EXP, scale=0.125)
                        nc.vector.tensor_mul(
                            pt[:, c0:c0 + P], pt[:, c0:c0 + P], tri_sb[:])
                else:
                    nc.scalar.activation(pt[:], stp[:], EXP, scale=0.125)
                sub0 = ka // P
                for sub in range(sub0, 4):
                    last = si == 4 * qb + sub
                    for hh in range(2):
                        h = 2 * pr + hh
                        first = pr == 0 and hh == 0 and si == 0 and sub % 2 == 0
                        pslice = pt[:, hh * TBLK + sub * P:
                                    hh * TBLK + (sub + 1) * P]
                        nc.tensor.matmul(
                            att[sub // 2][:, (sub % 2) * HD + h * HS:
                                          (sub % 2) * HD + (h + 1) * HS],
                            lhsT=pslice, rhs=v_sb[si][:, h * HS:(h + 1) * HS],
                            start=first, stop=last, skip_group_check=True,
                        )
                        dfirst = pr == 0 and hh == 0 and si == 0 and sub == 0
                        dcol = pr * 8 + sub * 2 + hh
                        nc.tensor.matmul(
                            den[:, dcol:dcol + 1],
                            lhsT=pslice, rhs=ones_sb[:],
                            start=dfirst, stop=last, skip_group_check=True,
                        )
                # normalize q-subtiles whose denominators just completed
                # (all but the last, which emit_finish_pair handles)
                if si >= 4 * qb and si - 4 * qb < 3:
                    emit_norm_sub(qb, pr, si - 4 * qb)
                yield

        # ------- normalize (per-sub, early) + AllGather for one pair -------
        # att stays in [q, d] layout end-to-end on this side; the transpose
        # to [d, t] happens in the lh load from DRAM via the XBAR.
        lh_of = {}
        asb_of = {}

        def emit_norm_sub(qb, pr, sub):
            """Reciprocal + normalize one q-subtile as soon as its
            denominators are complete (after s-tile si = 4qb+sub)."""
            att, den = att_of[qb], den_of[qb]
            if sub == 0:
                asb_of[(qb, pr)] = asp.tile(
                    [P, 4 * P], BF16, tag=f"asb{pr}", name=f"asb{qb}_{pr}")
            rec = rp.tile([P, 2], F32, tag="rec", name=f"rec{qb}_{pr}_{sub}")
            dcol = pr * 8 + sub * 2
            nc.vector.reciprocal(rec[:], den[:, dcol:dcol + 2])
            in0 = att[sub // 2][:, (sub % 2) * HD + pr * P:
                                (sub % 2) * HD + (pr + 1) * P]
            nc.vector.tensor_tensor(
                asb_of[(qb, pr)][:, sub * P:(sub + 1) * P]
                .rearrange("p (h d) -> p h d", h=2),
                in0.rearrange("p (h d) -> p h d", h=2),
                rec[:].broadcast_to([P, 2, HS]),
                MULT,
            )

        def emit_finish_pair(qb, pr):
            emit_norm_sub(qb, pr, 3)
            a_sb = asb_of[(qb, pr)]
            # AllGather this pair's att [512 t, 128 d] (t-major rows)
            ag_in = dramp.tile([4 * P, P], BF16, tag=f"agin{qb}_{pr}")
            nc.sync.dma_start(
                ag_in[:].rearrange("(s q) d -> q s d", q=P),
                a_sb[:].rearrange("q (s d) -> q s d", s=4),
            )
            ag_out = dramp.tile([GROUPS * 4 * P, P], BF16,
                                tag=f"agout{qb}_{pr}")
            if with_collective:
                nc.gpsimd.collective_compute(
                    "AllGather",
                    mybir.AluOpType.bypass,
                    replica_groups=REPLICA_GROUPS,
                    ins=[ag_in[:].opt()],
                    outs=[ag_out[:].opt()],
                )
            else:  # timing/sim variant: fake the AG with local DMA copies
                nc.sync.dma_start(
                    ag_out[:].rearrange("(g t) d -> t g d", t=4 * P),
                    ag_in[:].rearrange("(g t) d -> t g d", g=1)
                    .broadcast_to([4 * P, GROUPS, P]),
                )
            # transpose-load the gathered chunks: lh[g] = [128 d, 512 t]
            lhs = []
            for g in range(GROUPS):
                lh_t = lhp.tile([P, TBLK], BF16, tag=f"lh{pr}_{g}",
                                name=f"lh{qb}_{pr}_{g}")
                nc.sync.dma_start_transpose(
                    lh_t[:], ag_out[g * 4 * P:(g + 1) * 4 * P, :])
                lhs.append(lh_t)
            lh_of[(qb, pr)] = lhs

        # ---------------- stage-3: out^T projection ----------------
        def emit_oproj(qb, et, pool=None, tag="small",
                       korder=(0, 2, 4, 6, 1, 3, 5, 7), split_out=False):
            pool = pool or psC
            op = pool.tile([P, TBLK], F32, tag=tag, name=f"op{qb}_{et}")
            for j, k in enumerate(korder):
                nc.tensor.matmul(
                    op[:],
                    lhsT=w_sb["wo"][:, k * ES + et * P:k * ES + (et + 1) * P],
                    rhs=lh_of[(qb, k % 2)][k // 2][:],
                    start=(j == 0), stop=(j == NCT - 1),
                )
            o_sb = osp.tile([P, TBLK], F32, tag="osb", name=f"osb{qb}_{et}")
            nh = 2 if split_out else 1
            for h in range(nh):
                sl = slice(h * TBLK // nh, (h + 1) * TBLK // nh)
                nc.vector.tensor_scalar_add(
                    o_sb[:, sl], op[:, sl], bo_sb[:, et:et + 1])
                nc.sync.dma_start(
                    outT[et * P:(et + 1) * P,
                         qb * TBLK + sl.start:qb * TBLK + sl.stop],
                    o_sb[:, sl])

        def oproj_fillers(qb):
            return [lambda et=et, q=qb: emit_oproj(q, et) for et in range(2)]

        # ---------------- emission schedule ----------------
        load_w("wq", wq, HD)
        load_w("wk", wk, HD)
        for ci in range(NCT):
            load_x_tb0(ci)
        load_w("wv", wv, HD)
        load_consts()
        # startup chunks spread across the (still free) PSUM banks so they
        # don't serialize on the single shared bank
        emit_qk_proj(0, 0, 0, psC, "small")
        emit_qk_proj(0, 0, 1, psD, "den")
        emit_v_proj(0, psA, "att0")
        emit_v_proj(1, psA, "att1")
        emit_v_proj(2, psC, "small")
        emit_v_proj(3, psD, "den")
        for ci in range(NCT):
            load_x_rest(ci)
        load_w("wo", wo, ES)
        nc.sync.dma_start(bo_sb[:], bo2[:])

        def drive_pair(qb, pr, vfill, fillers, stride, off=0):
            ctr = 0
            for _ in emit_headpair(qb, pr):
                ctr += 1
                if vfill and ctr % 2 == 1:
                    vfill.pop(0)()
                elif (fillers and ctr > off
                      and (ctr - off) % stride == 0):
                    fillers.pop(0)()

        # fillers per qb: v for qb's own diagonal on odd units (vfill),
        # stage-1 for qb+1 and delayed out-projections strided (fillers)
        qkp1 = [lambda w=w: emit_qk_proj(0, 1, w) for w in range(2)]
        plan = {
            0: ([], qkp1 + qk_chunks(1)),
            1: (v_chunks(1), qk_chunks(2)),
            2: (v_chunks(2), qk_chunks(3)),
            3: (v_chunks(3), oproj_fillers(0) + oproj_fillers(1)),
        }
        for qb in range(NTB):
            new_qb_psum(qb)
            vfill, fillers = plan[qb]
            vfill, fillers = list(vfill), list(fillers)
            ns = 4 * (qb + 1)
            stride = max(1, (2 * ns) // max(1, len(fillers) + 1))
            drive_pair(qb, 0, vfill, fillers, stride)
            emit_finish_pair(qb, 0)
            drive_pair(qb, 1, vfill, fillers, stride)
            while vfill:
                vfill.pop(0)()
            while fillers:
                fillers.pop(0)()
            emit_finish_pair(qb, 1)

        # tail: out-projections of qb2 (its lh landed long ago — keeps PE
        # busy and hot while qb3-pair1's AllGather chain is in flight),
        # then qb3's. The retired den bank doubles as a second accumulator
        # so the two e-tiles run in parallel instead of serializing on psC.
        emit_oproj(2, 0, psC, "small")
        emit_oproj(2, 1, psD, "den")
        # keep the PE p-state hot while the final AllGather chain is in
        # flight: harmless matmuls into the retired scores banks
        for wi in range(24):
            warm = ps2.tile([P, 2 * TBLK], F32, tag="st2", name=f"warm{wi}")
            for half in range(2):
                nc.tensor.matmul(
                    warm[:, half * TBLK:(half + 1) * TBLK],
                    lhsT=w_sb["wq"][:, 0:P], rhs=w_sb["wq"][:, 0:TBLK],
                    start=True, stop=True,
                )
        late = (1, 3, 5, 7, 0, 2, 4, 6)
        emit_oproj(3, 0, psC, "small", korder=late, split_out=True)
        emit_oproj(3, 1, psD, "den", korder=late, split_out=True)

    nc.compile()
    return nc


_NC_CACHE = {}


def _get_nc(with_collective=True):
    key = with_collective
    if key not in _NC_CACHE:
        _NC_CACHE[key] = build_nc(with_collective)
    return _NC_CACHE[key]


def make_in_maps(x, Wq, Wk, Wv, Wo, bo):
    tri = np.ascontiguousarray(np.triu(np.ones((P, P), np.float32))).astype(BF)
    onescol = np.ones((P, 1), dtype=BF)
    in_maps = []
    for c in range(N_CORES):
        b, g = c // GROUPS, c % GROUPS
        hs_ = slice(g * HPG, (g + 1) * HPG)
        bo_sl = bo[g * ES:(g + 1) * ES].astype(np.float32)
        in_maps.append({
            "xT": np.ascontiguousarray(x[b].T).astype(BF),
            "wq": np.ascontiguousarray(
                Wq[hs_].transpose(1, 0, 2).reshape(C, HD)).astype(BF),
            "wk": np.ascontiguousarray(
                Wk[hs_].transpose(1, 0, 2).reshape(C, HD)).astype(BF),
            "wv": np.ascontiguousarray(
                Wv[hs_].transpose(1, 0, 2).reshape(C, HD)).astype(BF),
            "wo": np.ascontiguousarray(Wo[:, g * ES:(g + 1) * ES]).astype(BF),
            "bo2": np.ascontiguousarray(bo_sl.reshape(2, P).T),
            "trineg": tri,
            "onescol": onescol,
        })
    return in_maps


def kernel(x, Wq, Wk, Wv, Wo, bo):
    x = np.asarray(x, dtype=np.float32)
    Wq = np.asarray(Wq, dtype=np.float32)
    Wk = np.asarray(Wk, dtype=np.float32)
    Wv = np.asarray(Wv, dtype=np.float32)
    Wo = np.asarray(Wo, dtype=np.float32)
    bo = np.asarray(bo, dtype=np.float32)

    nc = _get_nc(with_collective=True)
    in_maps = make_in_maps(x, Wq, Wk, Wv, Wo, bo)
    res = run_bass_kernel_spmd(nc, in_maps, core_ids=list(range(N_CORES)))

    out = np.empty((B, T, E), dtype=np.float32)
    for c in range(N_CORES):
        b, g = c // GROUPS, c % GROUPS
        out[b, :, g * ES:(g + 1) * ES] = res.results[c]["outT"].T
    return out


# revision 5
# speedup vs baseline: 1.2662x; 1.0032x over previous
"""Multi-head causal self-attention on 8 TRN2 NeuronCores — v2 (bf16).

Problem (nn_MultiHeadAttention): B=2, T=2048, C=1024, H=16 heads, hs=64.

Sharding: core c -> (batch b = c//4, head-group g = c%4, 4 heads each).
AllGather of normalized attention outputs across the 4 cores of a batch
(replica groups [0-3], [4-7]); each core computes a disjoint 256-column
slice of the output projection, TRANSPOSED (out^T [256, 2048]); host
transposes + concatenates.

v2 design (cost-model-driven; matmul cost = out free-size x cycles/row):
- All matmul I/O in bf16 (1 cycle/row at any size; halves SBUF/DMA bytes).
  PSUM accumulation stays f32; measured rel err ~3.5e-3 vs the 2e-2 gate.
- Scores S^T [s,q] per head pair, rhs q-range trimmed to the causal part;
  the diagonal 128-block is masked by a DVE triu-multiply on the exp'd
  probabilities (bf16, 2x DVE mode).
- AV in [q,d] layout: att[q,h] accumulated per 128-q-subtile with
  lhsT = pt-slice (cost 64+1 per matmul instead of 512). Softmax
  denominators via N=1 matmuls (rhs=ones) into a separate PSUM bank.
  Normalization is a [128,2] reciprocal + stride-0-broadcast multiply per
  subtile, emitted EARLY (as soon as its denominators complete mid-loop).
- The pair's att [512 t, 128 d] is AllGathered t-major; the transpose to
  [d, t] for the output projection happens in the gather readback via two
  XBAR DMA-transpose loads (no PE/PSUM/DVE involvement).
- Output projection computes out^T [e,t] (lhsT=Wo tile, rhs=AllGathered
  att^T), so the bias is per-partition and folds into the PSUM->SBUF copy.
- PSUM bank discipline: matmul start=True zeroes a whole 2KB bank, so each
  bank gets exactly one start and slice-groups accumulate with start=False.
  Banks: scores ping-pong 2x[128,1024] (4), att accum 2x[128,512] (2),
  denominators [128,512] (1), shared small pool (1) = 8. GPSIMD cannot
  touch PSUM, so PSUM evacuation rides on DVE.
- DMAs batched (one per weight matrix, per x row-tile, per lh gather);
  x loads split tb0-first so stage-1 starts early.
- Tail: final out-projections order lh operands latest-dep-first (stops
  the tile scheduler hoisting them ahead of the last s-loop and
  head-of-line blocking PE), warm-up matmuls hold the PE p-state at full
  clock across the final AllGather latency, qb2's out-projection is
  deferred into the same window, and the last stores split 384/128 so the
  final DMA chain is short.
"""

import numpy as np
import ml_dtypes
from contextlib import ExitStack

import concourse.bass as bass
import concourse.mybir as mybir
import concourse.tile as tile
from concourse import bacc
from concourse.bass_utils import run_bass_kernel_spmd

F32 = mybir.dt.float32
BF16 = mybir.dt.bfloat16
BF = ml_dtypes.bfloat16
EXP = mybir.ActivationFunctionType.Exp
MULT = mybir.AluOpType.mult

N_CORES = 8
B = 2
T = 2048
C = 1024
NH = 16
HS = 64
E = 1024
GROUPS = 4
HPG = NH // GROUPS   # 4 heads per core
ES = E // GROUPS     # 256 output columns per core
HD = HPG * HS        # 256 local attention-output rows

P = 128
TBLK = 512
NTB = T // TBLK      # 4
NCT = C // P         # 8
NST = T // P         # 16

REPLICA_GROUPS = [[0, 1, 2, 3], [4, 5, 6, 7]]


def build_nc(with_collective=True):
    nc = bacc.Bacc(
        "TRN2", target_bir_lowering=False, debug=False, num_devices=N_CORES
    )

    xT = nc.dram_tensor("xT", [C, T], BF16, kind="ExternalInput").ap()
    wq = nc.dram_tensor("wq", [C, HD], BF16, kind="ExternalInput").ap()
    wk = nc.dram_tensor("wk", [C, HD], BF16, kind="ExternalInput").ap()
    wv = nc.dram_tensor("wv", [C, HD], BF16, kind="ExternalInput").ap()
    wo = nc.dram_tensor("wo", [E, ES], BF16, kind="ExternalInput").ap()
    bo2 = nc.dram_tensor("bo2", [P, 2], F32, kind="ExternalInput").ap()
    trineg = nc.dram_tensor("trineg", [P, P], BF16, kind="ExternalInput").ap()
    onescol = nc.dram_tensor("onescol", [P, 1], BF16, kind="ExternalInput").ap()
    outT = nc.dram_tensor("outT", [ES, T], F32, kind="ExternalOutput").ap()

    with tile.TileContext(nc) as tc, ExitStack() as ctx:
        wp = ctx.enter_context(tc.tile_pool(name="wp", bufs=1))
        xp = ctx.enter_context(tc.tile_pool(name="xp", bufs=1))
        qkp = ctx.enter_context(tc.tile_pool(name="qkp", bufs=1))
        vp = ctx.enter_context(tc.tile_pool(name="vp", bufs=1))
        ptp = ctx.enter_context(tc.tile_pool(name="ptp", bufs=8))
        asp = ctx.enter_context(tc.tile_pool(name="asp", bufs=8))
        atp = ctx.enter_context(tc.tile_pool(name="atp", bufs=4))
        lhp = ctx.enter_context(tc.tile_pool(name="lhp", bufs=6))
        rp = ctx.enter_context(tc.tile_pool(name="rp", bufs=8))
        osp = ctx.enter_context(tc.tile_pool(name="osp", bufs=4))
        # PSUM: 8 banks. ps2 = scores ping-pong (2x2 banks), psA = att
        # accumulators (2x1), psD = denominators (1), psC = small shared (1).
        ps2 = ctx.enter_context(tc.tile_pool(name="ps2", bufs=2, space="PSUM"))
        psA = ctx.enter_context(tc.tile_pool(name="psA", bufs=1, space="PSUM"))
        psD = ctx.enter_context(tc.tile_pool(name="psD", bufs=1, space="PSUM"))
        psC = ctx.enter_context(tc.tile_pool(name="psC", bufs=1, space="PSUM"))
        dramp = ctx.enter_context(tc.tile_pool(name="dramp", bufs=1, space="DRAM"))

        # ---- constants (tiles; DMAs issued after the hot-path loads) ----
        tri_sb = wp.tile([P, P], BF16, tag="tri")
        ones_sb = wp.tile([P, 1], BF16, tag="ones")
        bo_sb = wp.tile([P, 2], F32, tag="bo")

        def load_consts():
            nc.sync.dma_start(tri_sb[:], trineg[:])
            nc.sync.dma_start(ones_sb[:], onescol[:])

        # ---- big weight tiles, one DMA each: [128, 8*width] ----
        w_sb = {}

        def load_w(name, dram, width):
            t_ = wp.tile([P, NCT * width], BF16, tag=name)
            nc.sync.dma_start(
                t_[:].rearrange("p (c e) -> p c e", c=NCT),
                dram.rearrange("(c p) e -> p c e", p=P),
            )
            w_sb[name] = t_

        # ---- x row-tiles: tb0 chunk first (unblocks stage-1 fast), rest
        # loaded in a second wave ----
        x_sb = []

        def load_x_tb0(ci):
            t_ = xp.tile([P, T], BF16, tag=f"x{ci}", name=f"x{ci}")
            nc.sync.dma_start(t_[:, 0:TBLK], xT[ci * P:(ci + 1) * P, 0:TBLK])
            x_sb.append(t_)

        def load_x_rest(ci):
            nc.sync.dma_start(
                x_sb[ci][:, TBLK:T], xT[ci * P:(ci + 1) * P, TBLK:T])

        # merged Q^T/K^T per head pair: col = tb*1024 + which*512 + t_local
        qkt = [qkp.tile([P, 2 * T], BF16, tag=f"qk{p_}", name=f"qk{p_}")
               for p_ in range(2)]

        def qt_slice(pr, hh, t0, tn):
            tb, tl = t0 // TBLK, t0 % TBLK
            base = tb * 1024 + tl
            return qkt[pr][hh * HS:(hh + 1) * HS, base:base + tn]

        def kt_slice(pr, hh, s0, sn):
            tb, sl = s0 // TBLK, s0 % TBLK
            base = tb * 1024 + TBLK + sl
            return qkt[pr][hh * HS:(hh + 1) * HS, base:base + sn]

        v_sb = [vp.tile([P, HD], BF16, tag=f"v{st}", name=f"v{st}")
                for st in range(NST)]

        # ---------------- stage-1 pieces ----------------
        def emit_qk_proj(tb, pr, which, pool=None, ptag="small"):
            pool = pool or psC
            ps_ = pool.tile([P, TBLK], F32, tag=ptag,
                            name=f"qkps{tb}_{pr}_{which}")
            wn = "wq" if which == 0 else "wk"
            for ci in range(NCT):
                nc.tensor.matmul(
                    ps_[:],
                    lhsT=w_sb[wn][:, ci * HD + pr * P:ci * HD + (pr + 1) * P],
                    rhs=x_sb[ci][:, tb * TBLK:(tb + 1) * TBLK],
                    start=(ci == 0), stop=(ci == NCT - 1),
                )
            base = tb * 1024 + which * TBLK
            nc.vector.tensor_copy(qkt[pr][:, base:base + TBLK], ps_[:])

        def emit_v_proj(st, pool=None, ptag="small"):
            pool = pool or psC
            ps_ = pool.tile([P, TBLK], F32, tag=ptag, name=f"vps{st}")
            for ci in range(NCT):
                nc.tensor.matmul(
                    ps_[:, 0:HD],
                    lhsT=x_sb[ci][:, st * P:(st + 1) * P],
                    rhs=w_sb["wv"][:, ci * HD:(ci + 1) * HD],
                    start=(ci == 0), stop=(ci == NCT - 1),
                )
            nc.vector.tensor_copy(v_sb[st][:], ps_[:, 0:HD])

        def qk_chunks(tb):
            return [lambda tb=tb, pr=pr, w=w: emit_qk_proj(tb, pr, w)
                    for pr in range(2) for w in range(2)]

        def v_chunks(tb):
            return [lambda st=st: emit_v_proj(st)
                    for st in range(4 * tb, 4 * tb + 4)]

        # ------- stage-2: s-loop of one head pair of one t-block ------
        att_of = {}   # qb -> [A01, A23] psum tiles [128, 512]
        den_of = {}   # qb -> psum tile [128, 512] (cols pr*8 + sub*2 + hh)

        def new_qb_psum(qb):
            att_of[qb] = [
                psA.tile([P, 4 * P], F32, tag=f"att{b_}", name=f"att{qb}_{b_}")
                for b_ in range(2)
            ]
            den_of[qb] = psD.tile([P, TBLK], F32, tag="den", name=f"den{qb}")

        def emit_headpair(qb, pr):
            """Scores + exp + AV/denominator accumulation for heads
            (2pr, 2pr+1). Yields once per s-tile for the filler driver."""
            t0 = qb * TBLK
            ns = 4 * (qb + 1)
            att, den = att_of[qb], den_of[qb]
            for si in range(ns):
                diag = si * P >= t0
                ka = si * P - t0 if diag else 0
                stp = ps2.tile([P, 2 * TBLK], F32, tag="st2",
                               name=f"st{qb}_{pr}_{si}")
                for hh in range(2):
                    c0 = hh * TBLK + ka
                    nc.tensor.matmul(
                        stp[:, c0:(hh + 1) * TBLK],
                        lhsT=kt_slice(pr, hh, si * P, P),
                        rhs=qt_slice(pr, hh, t0 + ka, TBLK - ka),
                        start=True, stop=True,
                    )
                pt = ptp.tile([P, 2 * TBLK], BF16, tag="pt",
                              name=f"pt{qb}_{pr}_{si}")
                if diag:
                    for hh in range(2):
                        c0 = hh * TBLK + ka
                        nc.scalar.activation(
                            pt[:, c0:(hh + 1) * TBLK],
                            stp[:, c0:(hh + 1) * TBLK], EXP, scale=0.125)
                        nc.vector.tensor_mul(
                            pt[:, c0:c0 + P], pt[:, c0:c0 + P], tri_sb[:])
                else:
                    nc.scalar.activation(pt[:], stp[:], EXP, scale=0.125)
                sub0 = ka // P
                for sub in range(sub0, 4):
                    last = si == 4 * qb + sub
                    for hh in range(2):
                        h = 2 * pr + hh
                        first = pr == 0 and hh == 0 and si == 0 and sub % 2 == 0
                        pslice = pt[:, hh * TBLK + sub * P:
                                    hh * TBLK + (sub + 1) * P]
                        nc.tensor.matmul(
                            att[sub // 2][:, (sub % 2) * HD + h * HS:
                                          (sub % 2) * HD + (h + 1) * HS],
                            lhsT=pslice, rhs=v_sb[si][:, h * HS:(h + 1) * HS],
                            start=first, stop=last, skip_group_check=True,
                        )
                        dfirst = pr == 0 and hh == 0 and si == 0 and sub == 0
                        dcol = pr * 8 + sub * 2 + hh
                        nc.tensor.matmul(
                            den[:, dcol:dcol + 1],
                            lhsT=pslice, rhs=ones_sb[:],
                            start=dfirst, stop=last, skip_group_check=True,
                        )
                # normalize q-subtiles whose denominators just completed
                # (all but the last, which emit_finish_pair handles)
                if si >= 4 * qb and si - 4 * qb < 3:
                    emit_norm_sub(qb, pr, si - 4 * qb)
                yield

        # ------- normalize (per-sub, early) + AllGather for one pair -------
        # att stays in [q, d] layout end-to-end on this side; the transpose
        # to [d, t] happens in the lh load from DRAM via the XBAR.
        lh_of = {}
        asb_of = {}

        def emit_norm_sub(qb, pr, sub):
            """Reciprocal + normalize one q-subtile as soon as its
            denominators are complete (after s-tile si = 4qb+sub)."""
            att, den = att_of[qb], den_of[qb]
            if sub == 0:
                asb_of[(qb, pr)] = asp.tile(
                    [P, 4 * P], BF16, tag=f"asb{pr}", name=f"asb{qb}_{pr}")
            rec = rp.tile([P, 2], F32, tag="rec", name=f"rec{qb}_{pr}_{sub}")
            dcol = pr * 8 + sub * 2
            nc.vector.reciprocal(rec[:], den[:, dcol:dcol + 2])
            in0 = att[sub // 2][:, (sub % 2) * HD + pr * P:
                                (sub % 2) * HD + (pr + 1) * P]
            nc.vector.tensor_tensor(
                asb_of[(qb, pr)][:, sub * P:(sub + 1) * P]
                .rearrange("p (h d) -> p h d", h=2),
                in0.rearrange("p (h d) -> p h d", h=2),
                rec[:].broadcast_to([P, 2, HS]),
                MULT,
            )

        def emit_finish_pair(qb, pr):
            emit_norm_sub(qb, pr, 3)
            a_sb = asb_of[(qb, pr)]
            # AllGather this pair's att [512 t, 128 d] (t-major rows)
            ag_in = dramp.tile([4 * P, P], BF16, tag=f"agin{qb}_{pr}")
            nc.sync.dma_start(
                ag_in[:].rearrange("(s q) d -> q s d", q=P),
                a_sb[:].rearrange("q (s d) -> q s d", s=4),
            )
            ag_out = dramp.tile([GROUPS * 4 * P, P], BF16,
                                tag=f"agout{qb}_{pr}")
            if with_collective:
                nc.gpsimd.collective_compute(
                    "AllGather",
                    mybir.AluOpType.bypass,
                    replica_groups=REPLICA_GROUPS,
                    ins=[ag_in[:].opt()],
                    outs=[ag_out[:].opt()],
                )
            else:  # timing/sim variant: fake the AG with local DMA copies
                nc.sync.dma_start(
                    ag_out[:].rearrange("(g t) d -> t g d", t=4 * P),
                    ag_in[:].rearrange("(g t) d -> t g d", g=1)
                    .broadcast_to([4 * P, GROUPS, P]),
                )
            # transpose-load the gathered chunks two groups at a time:
            # lh half = [128 d, 1024 (g-major t)]
            lhs = []
            for gh in range(2):
                lh_t = lhp.tile([P, 2 * TBLK], BF16, tag=f"lh{pr}_{gh}",
                                name=f"lh{qb}_{pr}_{gh}")
                nc.sync.dma_start_transpose(
                    lh_t[:], ag_out[gh * 8 * P:(gh + 1) * 8 * P, :])
                lhs.append(lh_t)
            lh_of[(qb, pr)] = [
                lhs[g // 2][:, (g % 2) * TBLK:(g % 2 + 1) * TBLK]
                for g in range(GROUPS)
            ]

        # ---------------- stage-3: out^T projection ----------------
        def emit_oproj(qb, et, pool=None, tag="small",
                       korder=(0, 2, 4, 6, 1, 3, 5, 7), split_out=False):
            pool = pool or psC
            op = pool.tile([P, TBLK], F32, tag=tag, name=f"op{qb}_{et}")
            for j, k in enumerate(korder):
                nc.tensor.matmul(
                    op[:],
                    lhsT=w_sb["wo"][:, k * ES + et * P:k * ES + (et + 1) * P],
                    rhs=lh_of[(qb, k % 2)][k // 2],
                    start=(j == 0), stop=(j == NCT - 1),
                )
            o_sb = osp.tile([P, TBLK], F32, tag="osb", name=f"osb{qb}_{et}")
            # asymmetric split: big piece first, small final piece so the
            # last store's latency chain is short
            cuts = (0, 384, TBLK) if split_out else (0, TBLK)
            for h in range(len(cuts) - 1):
                sl = slice(cuts[h], cuts[h + 1])
                nc.vector.tensor_scalar_add(
                    o_sb[:, sl], op[:, sl], bo_sb[:, et:et + 1])
                nc.sync.dma_start(
                    outT[et * P:(et + 1) * P,
                         qb * TBLK + sl.start:qb * TBLK + sl.stop],
                    o_sb[:, sl])

        def oproj_fillers(qb):
            return [lambda et=et, q=qb: emit_oproj(q, et) for et in range(2)]

        # ---------------- emission schedule ----------------
        load_w("wq", wq, HD)
        load_w("wk", wk, HD)
        for ci in range(NCT):
            load_x_tb0(ci)
        load_w("wv", wv, HD)
        load_consts()
        # startup chunks spread across the (still free) PSUM banks so they
        # don't serialize on the single shared bank
        emit_qk_proj(0, 0, 0, psC, "small")
        emit_qk_proj(0, 0, 1, psD, "den")
        emit_v_proj(0, psA, "att0")
        emit_v_proj(1, psA, "att1")
        emit_v_proj(2, psC, "small")
        emit_v_proj(3, psD, "den")
        for ci in range(NCT):
            load_x_rest(ci)
        load_w("wo", wo, ES)
        nc.sync.dma_start(bo_sb[:], bo2[:])

        def drive_pair(qb, pr, vfill, fillers, stride, off=0):
            ctr = 0
            for _ in emit_headpair(qb, pr):
                ctr += 1
                if vfill and ctr % 2 == 1:
                    vfill.pop(0)()
                elif (fillers and ctr > off
                      and (ctr - off) % stride == 0):
                    fillers.pop(0)()

        # fillers per qb: v for qb's own diagonal on odd units (vfill),
        # stage-1 for qb+1 and delayed out-projections strided (fillers)
        qkp1 = [lambda w=w: emit_qk_proj(0, 1, w) for w in range(2)]
        plan = {
            0: ([], qkp1 + qk_chunks(1)),
            1: (v_chunks(1), qk_chunks(2)),
            2: (v_chunks(2), qk_chunks(3)),
            3: (v_chunks(3), oproj_fillers(0) + oproj_fillers(1)),
        }
        for qb in range(NTB):
            new_qb_psum(qb)
            vfill, fillers = plan[qb]
            vfill, fillers = list(vfill), list(fillers)
            ns = 4 * (qb + 1)
            stride = max(1, (2 * ns) // max(1, len(fillers) + 1))
            drive_pair(qb, 0, vfill, fillers, stride)
            emit_finish_pair(qb, 0)
            drive_pair(qb, 1, vfill, fillers, stride)
            while vfill:
                vfill.pop(0)()
            while fillers:
                fillers.pop(0)()
            emit_finish_pair(qb, 1)

        # tail: out-projections of qb2 (its lh landed long ago — keeps PE
        # busy and hot while qb3-pair1's AllGather chain is in flight),
        # then qb3's. The retired den bank doubles as a second accumulator
        # so the two e-tiles run in parallel instead of serializing on psC.
        emit_oproj(2, 0, psC, "small")
        emit_oproj(2, 1, psD, "den")
        # keep the PE p-state hot while the final AllGather chain is in
        # flight: harmless matmuls into the retired scores banks
        for wi in range(24):
            warm = ps2.tile([P, 2 * TBLK], F32, tag="st2", name=f"warm{wi}")
            for half in range(2):
                nc.tensor.matmul(
                    warm[:, half * TBLK:(half + 1) * TBLK],
                    lhsT=w_sb["wq"][:, 0:P], rhs=w_sb["wq"][:, 0:TBLK],
                    start=True, stop=True,
                )
        late = (1, 3, 5, 7, 0, 2, 4, 6)
        emit_oproj(3, 0, psC, "small", korder=late, split_out=True)
        emit_oproj(3, 1, psD, "den", korder=late, split_out=True)

    nc.compile()
    return nc


_NC_CACHE = {}


def _get_nc(with_collective=True):
    key = with_collective
    if key not in _NC_CACHE:
        _NC_CACHE[key] = build_nc(with_collective)
    return _NC_CACHE[key]


def make_in_maps(x, Wq, Wk, Wv, Wo, bo):
    tri = np.ascontiguousarray(np.triu(np.ones((P, P), np.float32))).astype(BF)
    onescol = np.ones((P, 1), dtype=BF)
    in_maps = []
    for c in range(N_CORES):
        b, g = c // GROUPS, c % GROUPS
        hs_ = slice(g * HPG, (g + 1) * HPG)
        bo_sl = bo[g * ES:(g + 1) * ES].astype(np.float32)
        in_maps.append({
            "xT": np.ascontiguousarray(x[b].T).astype(BF),
            "wq": np.ascontiguousarray(
                Wq[hs_].transpose(1, 0, 2).reshape(C, HD)).astype(BF),
            "wk": np.ascontiguousarray(
                Wk[hs_].transpose(1, 0, 2).reshape(C, HD)).astype(BF),
            "wv": np.ascontiguousarray(
                Wv[hs_].transpose(1, 0, 2).reshape(C, HD)).astype(BF),
            "wo": np.ascontiguousarray(Wo[:, g * ES:(g + 1) * ES]).astype(BF),
            "bo2": np.ascontiguousarray(bo_sl.reshape(2, P).T),
            "trineg": tri,
            "onescol": onescol,
        })
    return in_maps


def kernel(x, Wq, Wk, Wv, Wo, bo):
    x = np.asarray(x, dtype=np.float32)
    Wq = np.asarray(Wq, dtype=np.float32)
    Wk = np.asarray(Wk, dtype=np.float32)
    Wv = np.asarray(Wv, dtype=np.float32)
    Wo = np.asarray(Wo, dtype=np.float32)
    bo = np.asarray(bo, dtype=np.float32)

    nc = _get_nc(with_collective=True)
    in_maps = make_in_maps(x, Wq, Wk, Wv, Wo, bo)
    res = run_bass_kernel_spmd(nc, in_maps, core_ids=list(range(N_CORES)))

    out = np.empty((B, T, E), dtype=np.float32)
    for c in range(N_CORES):
        b, g = c // GROUPS, c % GROUPS
        out[b, :, g * ES:(g + 1) * ES] = res.results[c]["outT"].T
    return out


# revision 7
# speedup vs baseline: 1.3189x; 1.0416x over previous
"""Multi-head causal self-attention on 8 TRN2 NeuronCores — v2 (bf16).

Problem (nn_MultiHeadAttention): B=2, T=2048, C=1024, H=16 heads, hs=64.

Sharding: core c -> (batch b = c//4, head-group g = c%4, 4 heads each).
AllGather of normalized attention outputs across the 4 cores of a batch
(replica groups [0-3], [4-7]); each core computes a disjoint 256-column
slice of the output projection, TRANSPOSED (out^T [256, 2048]); host
transposes + concatenates.

v2 design (cost-model-driven):
- All matmul I/O in bf16 (1 cycle/row at any size; halves SBUF/DMA bytes).
  PSUM accumulation stays f32; rel-err budget 2e-2 >> bf16 noise.
- Scores S^T [s,q] per head pair as before, but causal mask applied ON PE:
  an extra accumulate matmul (lhsT=I, rhs=tri(-32768)) onto the diagonal
  128-block, so exp needs no DVE mask multiply.
- AV in [q,d] layout: att[q,h] accumulated per 128-q-subtile with
  lhsT = pt-slice (cost 64/matmul instead of 512). Softmax denominator via
  N=1 matmuls (rhs=ones) into a separate PSUM bank. Normalization becomes a
  reciprocal [128,8] + stride-0-broadcast multiplies (cheap DVE ops).
- att^T for the collective produced by XBAR DMA transpose (no PE/PSUM).
- Output projection computes out^T [e,t] (lhsT=Wo tile, rhs=AllGathered
  att^T), so the bias is per-partition and folds into the PSUM->SBUF copy.
- PSUM bank discipline: matmul start=True zeroes a whole 2KB bank, so each
  bank gets exactly one start and slice-groups accumulate with start=False.
  Banks: scores ping-pong 2x[128,1024] (4), att accum 2x[128,512] (2),
  denominators [128,512] (1), shared small pool (1) = 8.
- DMAs batched (one per weight matrix, per x row-tile, per lh gather) to
  keep the serial HWDGE setup (~630ns each) off the critical path.
"""

import numpy as np
import ml_dtypes
from contextlib import ExitStack

import concourse.bass as bass
import concourse.mybir as mybir
import concourse.tile as tile
from concourse import bacc
from concourse.bass_utils import run_bass_kernel_spmd

F32 = mybir.dt.float32
BF16 = mybir.dt.bfloat16
BF = ml_dtypes.bfloat16
EXP = mybir.ActivationFunctionType.Exp
MULT = mybir.AluOpType.mult

N_CORES = 8
B = 2
T = 2048
C = 1024
NH = 16
HS = 64
E = 1024
GROUPS = 4
HPG = NH // GROUPS   # 4 heads per core
ES = E // GROUPS     # 256 output columns per core
HD = HPG * HS        # 256 local attention-output rows

P = 128
TBLK = 512
NTB = T // TBLK      # 4
NCT = C // P         # 8
NST = T // P         # 16

REPLICA_GROUPS = [[0, 1, 2, 3], [4, 5, 6, 7]]


def build_nc(with_collective=True):
    nc = bacc.Bacc(
        "TRN2", target_bir_lowering=False, debug=False, num_devices=N_CORES
    )

    xT = nc.dram_tensor("xT", [C, T], BF16, kind="ExternalInput").ap()
    wq = nc.dram_tensor("wq", [C, HD], BF16, kind="ExternalInput").ap()
    wk = nc.dram_tensor("wk", [C, HD], BF16, kind="ExternalInput").ap()
    wv = nc.dram_tensor("wv", [C, HD], BF16, kind="ExternalInput").ap()
    wo = nc.dram_tensor("wo", [E, ES], BF16, kind="ExternalInput").ap()
    bo2 = nc.dram_tensor("bo2", [P, 2], F32, kind="ExternalInput").ap()
    trineg = nc.dram_tensor("trineg", [P, P], BF16, kind="ExternalInput").ap()
    onescol = nc.dram_tensor("onescol", [P, 1], BF16, kind="ExternalInput").ap()
    outT = nc.dram_tensor("outT", [ES, T], F32, kind="ExternalOutput").ap()

    with tile.TileContext(nc) as tc, ExitStack() as ctx:
        wp = ctx.enter_context(tc.tile_pool(name="wp", bufs=1))
        xp = ctx.enter_context(tc.tile_pool(name="xp", bufs=1))
        qkp = ctx.enter_context(tc.tile_pool(name="qkp", bufs=1))
        vp = ctx.enter_context(tc.tile_pool(name="vp", bufs=1))
        ptp = ctx.enter_context(tc.tile_pool(name="ptp", bufs=8))
        asp = ctx.enter_context(tc.tile_pool(name="asp", bufs=8))
        atp = ctx.enter_context(tc.tile_pool(name="atp", bufs=4))
        lhp = ctx.enter_context(tc.tile_pool(name="lhp", bufs=6))
        rp = ctx.enter_context(tc.tile_pool(name="rp", bufs=8))
        osp = ctx.enter_context(tc.tile_pool(name="osp", bufs=4))
        # PSUM: 8 banks. ps2 = scores ping-pong (2x2 banks), psA = att
        # accumulators (2x1), psD = denominators (1), psC = small shared (1).
        ps2 = ctx.enter_context(tc.tile_pool(name="ps2", bufs=2, space="PSUM"))
        psA = ctx.enter_context(tc.tile_pool(name="psA", bufs=1, space="PSUM"))
        psD = ctx.enter_context(tc.tile_pool(name="psD", bufs=1, space="PSUM"))
        psC = ctx.enter_context(tc.tile_pool(name="psC", bufs=1, space="PSUM"))
        dramp = ctx.enter_context(tc.tile_pool(name="dramp", bufs=1, space="DRAM"))

        # ---- constants (tiles; DMAs issued after the hot-path loads) ----
        tri_sb = wp.tile([P, P], BF16, tag="tri")
        ones_sb = wp.tile([P, 1], BF16, tag="ones")
        bo_sb = wp.tile([P, 2], F32, tag="bo")

        def load_consts():
            nc.sync.dma_start(tri_sb[:], trineg[:])
            nc.sync.dma_start(ones_sb[:], onescol[:])

        # ---- big weight tiles, one DMA each: [128, 8*width] ----
        w_sb = {}

        def load_w(name, dram, width):
            t_ = wp.tile([P, NCT * width], BF16, tag=name)
            nc.sync.dma_start(
                t_[:].rearrange("p (c e) -> p c e", c=NCT),
                dram.rearrange("(c p) e -> p c e", p=P),
            )
            w_sb[name] = t_

        # ---- x row-tiles: tb0 chunk first (unblocks stage-1 fast), rest
        # loaded in a second wave ----
        x_sb = []

        def load_x_tb0(ci):
            t_ = xp.tile([P, T], BF16, tag=f"x{ci}", name=f"x{ci}")
            nc.sync.dma_start(t_[:, 0:TBLK], xT[ci * P:(ci + 1) * P, 0:TBLK])
            x_sb.append(t_)

        def load_x_rest(ci):
            nc.sync.dma_start(
                x_sb[ci][:, TBLK:T], xT[ci * P:(ci + 1) * P, TBLK:T])

        # merged Q^T/K^T per head pair: col = tb*1024 + which*512 + t_local
        qkt = [qkp.tile([P, 2 * T], BF16, tag=f"qk{p_}", name=f"qk{p_}")
               for p_ in range(2)]

        def qt_slice(pr, hh, t0, tn):
            tb, tl = t0 // TBLK, t0 % TBLK
            base = tb * 1024 + tl
            return qkt[pr][hh * HS:(hh + 1) * HS, base:base + tn]

        def kt_slice(pr, hh, s0, sn):
            tb, sl = s0 // TBLK, s0 % TBLK
            base = tb * 1024 + TBLK + sl
            return qkt[pr][hh * HS:(hh + 1) * HS, base:base + sn]

        v_sb = [vp.tile([P, HD], BF16, tag=f"v{st}", name=f"v{st}")
                for st in range(NST)]

        # ---------------- stage-1 pieces ----------------
        def emit_qk_proj(tb, pr, which, pool=None, ptag="small"):
            pool = pool or psC
            ps_ = pool.tile([P, TBLK], F32, tag=ptag,
                            name=f"qkps{tb}_{pr}_{which}")
            wn = "wq" if which == 0 else "wk"
            for ci in range(NCT):
                nc.tensor.matmul(
                    ps_[:],
                    lhsT=w_sb[wn][:, ci * HD + pr * P:ci * HD + (pr + 1) * P],
                    rhs=x_sb[ci][:, tb * TBLK:(tb + 1) * TBLK],
                    start=(ci == 0), stop=(ci == NCT - 1),
                )
            base = tb * 1024 + which * TBLK
            nc.vector.tensor_copy(qkt[pr][:, base:base + TBLK], ps_[:])

        def emit_v_proj(st, pool=None, ptag="small"):
            pool = pool or psC
            ps_ = pool.tile([P, TBLK], F32, tag=ptag, name=f"vps{st}")
            for ci in range(NCT):
                nc.tensor.matmul(
                    ps_[:, 0:HD],
                    lhsT=x_sb[ci][:, st * P:(st + 1) * P],
                    rhs=w_sb["wv"][:, ci * HD:(ci + 1) * HD],
                    start=(ci == 0), stop=(ci == NCT - 1),
                )
            nc.vector.tensor_copy(v_sb[st][:], ps_[:, 0:HD])

        def qk_chunks(tb):
            return [lambda tb=tb, pr=pr, w=w: emit_qk_proj(tb, pr, w)
                    for pr in range(2) for w in range(2)]

        def v_chunks(tb):
            return [lambda st=st: emit_v_proj(st)
                    for st in range(4 * tb, 4 * tb + 4)]

        # ------- stage-2: s-loop of one head pair of one t-block ------
        att_of = {}   # qb -> [A01, A23] psum tiles [128, 512]
        den_of = {}   # qb -> psum tile [128, 512] (cols pr*8 + sub*2 + hh)

        def new_qb_psum(qb):
            att_of[qb] = [
                psA.tile([P, 4 * P], F32, tag=f"att{b_}", name=f"att{qb}_{b_}")
                for b_ in range(2)
            ]
            den_of[qb] = psD.tile([P, TBLK], F32, tag="den", name=f"den{qb}")

        def emit_headpair(qb, pr):
            """Scores + exp + AV/denominator accumulation for heads
            (2pr, 2pr+1). Yields once per s-tile for the filler driver."""
            t0 = qb * TBLK
            ns = 4 * (qb + 1)
            att, den = att_of[qb], den_of[qb]
            for si in range(ns):
                diag = si * P >= t0
                ka = si * P - t0 if diag else 0
                stp = ps2.tile([P, 2 * TBLK], F32, tag="st2",
                               name=f"st{qb}_{pr}_{si}")
                for hh in range(2):
                    c0 = hh * TBLK + ka
                    nc.tensor.matmul(
                        stp[:, c0:(hh + 1) * TBLK],
                        lhsT=kt_slice(pr, hh, si * P, P),
                        rhs=qt_slice(pr, hh, t0 + ka, TBLK - ka),
                        start=True, stop=True,
                    )
                pt = ptp.tile([P, 2 * TBLK], BF16, tag="pt",
                              name=f"pt{qb}_{pr}_{si}")
                if diag:
                    # one strided exp + one strided mask over both heads:
                    # halves the per-op fixed overheads on the pacing chain
                    sv = stp[:].rearrange("p (h q) -> p h q", h=2)[:, :, ka:TBLK]
                    pv = pt[:].rearrange("p (h q) -> p h q", h=2)[:, :, ka:TBLK]
                    nc.scalar.activation(pv, sv, EXP, scale=0.125)
                    pm = pt[:].rearrange("p (h q) -> p h q", h=2)[:, :, ka:ka + P]
                    tri2 = (tri_sb[:].rearrange("p (h q) -> p h q", h=1)
                            .broadcast_to([P, 2, P]))
                    nc.vector.tensor_mul(pm, pm, tri2)
                else:
                    nc.scalar.activation(pt[:], stp[:], EXP, scale=0.125)
                sub0 = ka // P
                for sub in range(sub0, 4):
                    last = si == 4 * qb + sub
                    for hh in range(2):
                        h = 2 * pr + hh
                        first = pr == 0 and hh == 0 and si == 0 and sub % 2 == 0
                        pslice = pt[:, hh * TBLK + sub * P:
                                    hh * TBLK + (sub + 1) * P]
                        nc.tensor.matmul(
                            att[sub // 2][:, (sub % 2) * HD + h * HS:
                                          (sub % 2) * HD + (h + 1) * HS],
                            lhsT=pslice, rhs=v_sb[si][:, h * HS:(h + 1) * HS],
                            start=first, stop=last, skip_group_check=True,
                        )
                        dfirst = pr == 0 and hh == 0 and si == 0 and sub == 0
                        dcol = pr * 8 + sub * 2 + hh
                        nc.tensor.matmul(
                            den[:, dcol:dcol + 1],
                            lhsT=pslice, rhs=ones_sb[:],
                            start=dfirst, stop=last, skip_group_check=True,
                        )
                # normalize q-subtiles whose denominators just completed
                # (all but the last, which emit_finish_pair handles)
                if si >= 4 * qb and si - 4 * qb < 3:
                    emit_norm_sub(qb, pr, si - 4 * qb)
                yield

        # ------- normalize (per-sub, early) + AllGather for one pair -------
        # att stays in [q, d] layout end-to-end on this side; the transpose
        # to [d, t] happens in the lh load from DRAM via the XBAR.
        lh_of = {}
        asb_of = {}

        def emit_norm_sub(qb, pr, sub):
            """Reciprocal + normalize one q-subtile as soon as its
            denominators are complete (after s-tile si = 4qb+sub)."""
            att, den = att_of[qb], den_of[qb]
            if sub == 0:
                asb_of[(qb, pr)] = asp.tile(
                    [P, 4 * P], BF16, tag=f"asb{pr}", name=f"asb{qb}_{pr}")
            rec = rp.tile([P, 2], F32, tag="rec", name=f"rec{qb}_{pr}_{sub}")
            dcol = pr * 8 + sub * 2
            nc.vector.reciprocal(rec[:], den[:, dcol:dcol + 2])
            in0 = att[sub // 2][:, (sub % 2) * HD + pr * P:
                                (sub % 2) * HD + (pr + 1) * P]
            nc.vector.tensor_tensor(
                asb_of[(qb, pr)][:, sub * P:(sub + 1) * P]
                .rearrange("p (h d) -> p h d", h=2),
                in0.rearrange("p (h d) -> p h d", h=2),
                rec[:].broadcast_to([P, 2, HS]),
                MULT,
            )

        def emit_finish_pair(qb, pr):
            emit_norm_sub(qb, pr, 3)
            a_sb = asb_of[(qb, pr)]
            # AllGather this pair's att [512 t, 128 d] (t-major rows)
            ag_in = dramp.tile([4 * P, P], BF16, tag=f"agin{qb}_{pr}")
            nc.sync.dma_start(
                ag_in[:].rearrange("(s q) d -> q s d", q=P),
                a_sb[:].rearrange("q (s d) -> q s d", s=4),
            )
            ag_out = dramp.tile([GROUPS * 4 * P, P], BF16,
                                tag=f"agout{qb}_{pr}")
            if with_collective:
                nc.gpsimd.collective_compute(
                    "AllGather",
                    mybir.AluOpType.bypass,
                    replica_groups=REPLICA_GROUPS,
                    ins=[ag_in[:].opt()],
                    outs=[ag_out[:].opt()],
                )
            else:  # timing/sim variant: fake the AG with local DMA copies
                nc.sync.dma_start(
                    ag_out[:].rearrange("(g t) d -> t g d", t=4 * P),
                    ag_in[:].rearrange("(g t) d -> t g d", g=1)
                    .broadcast_to([4 * P, GROUPS, P]),
                )
            # transpose-load the gathered chunks two groups at a time:
            # lh half = [128 d, 1024 (g-major t)]
            lhs = []
            for gh in range(2):
                lh_t = lhp.tile([P, 2 * TBLK], BF16, tag=f"lh{pr}_{gh}",
                                name=f"lh{qb}_{pr}_{gh}")
                nc.sync.dma_start_transpose(
                    lh_t[:], ag_out[gh * 8 * P:(gh + 1) * 8 * P, :])
                lhs.append(lh_t)
            lh_of[(qb, pr)] = [
                lhs[g // 2][:, (g % 2) * TBLK:(g % 2 + 1) * TBLK]
                for g in range(GROUPS)
            ]

        # ---------------- stage-3: out^T projection ----------------
        def emit_oproj(qb, et, pool=None, tag="small",
                       korder=(0, 2, 4, 6, 1, 3, 5, 7), split_out=False):
            pool = pool or psC
            op = pool.tile([P, TBLK], F32, tag=tag, name=f"op{qb}_{et}")
            for j, k in enumerate(korder):
                nc.tensor.matmul(
                    op[:],
                    lhsT=w_sb["wo"][:, k * ES + et * P:k * ES + (et + 1) * P],
                    rhs=lh_of[(qb, k % 2)][k // 2],
                    start=(j == 0), stop=(j == NCT - 1),
                )
            o_sb = osp.tile([P, TBLK], F32, tag="osb", name=f"osb{qb}_{et}")
            # asymmetric split: big piece first, small final piece so the
            # last store's latency chain is short
            cuts = (0, 384, TBLK) if split_out else (0, TBLK)
            for h in range(len(cuts) - 1):
                sl = slice(cuts[h], cuts[h + 1])
                nc.vector.tensor_scalar_add(
                    o_sb[:, sl], op[:, sl], bo_sb[:, et:et + 1])
                nc.sync.dma_start(
                    outT[et * P:(et + 1) * P,
                         qb * TBLK + sl.start:qb * TBLK + sl.stop],
                    o_sb[:, sl])

        def oproj_fillers(qb):
            return [lambda et=et, q=qb: emit_oproj(q, et) for et in range(2)]

        # ---------------- emission schedule ----------------
        load_w("wq", wq, HD)
        load_w("wk", wk, HD)
        for ci in range(NCT):
            load_x_tb0(ci)
        load_w("wv", wv, HD)
        load_consts()
        # startup chunks spread across the (still free) PSUM banks so they
        # don't serialize on the single shared bank
        emit_qk_proj(0, 0, 0, psC, "small")
        emit_qk_proj(0, 0, 1, psD, "den")
        emit_v_proj(0, psA, "att0")
        emit_v_proj(1, psA, "att1")
        emit_v_proj(2, psC, "small")
        emit_v_proj(3, psD, "den")
        for ci in range(NCT):
            load_x_rest(ci)
        load_w("wo", wo, ES)
        nc.sync.dma_start(bo_sb[:], bo2[:])

        def drive_pair(qb, pr, vfill, fillers, stride, off=0):
            ctr = 0
            for _ in emit_headpair(qb, pr):
                ctr += 1
                if vfill and ctr % 2 == 1:
                    vfill.pop(0)()
                elif (fillers and ctr > off
                      and (ctr - off) % stride == 0):
                    fillers.pop(0)()

        # fillers per qb: v for qb's own diagonal on odd units (vfill),
        # stage-1 for qb+1 and delayed out-projections strided (fillers)
        qkp1 = [lambda w=w: emit_qk_proj(0, 1, w) for w in range(2)]
        plan = {
            0: ([], qkp1 + qk_chunks(1)),
            1: (v_chunks(1), qk_chunks(2)),
            2: (v_chunks(2), qk_chunks(3)),
            3: (v_chunks(3), oproj_fillers(0) + oproj_fillers(1)),
        }
        for qb in range(NTB):
            new_qb_psum(qb)
            vfill, fillers = plan[qb]
            vfill, fillers = list(vfill), list(fillers)
            ns = 4 * (qb + 1)
            stride = max(1, (2 * ns) // max(1, len(fillers) + 1))
            drive_pair(qb, 0, vfill, fillers, stride)
            emit_finish_pair(qb, 0)
            drive_pair(qb, 1, vfill, fillers, stride)
            while vfill:
                vfill.pop(0)()
            while fillers:
                fillers.pop(0)()
            emit_finish_pair(qb, 1)

        # tail: out-projections of qb2 (its lh landed long ago — keeps PE
        # busy and hot while qb3-pair1's AllGather chain is in flight),
        # then qb3's. The retired den bank doubles as a second accumulator
        # so the two e-tiles run in parallel instead of serializing on psC.
        emit_oproj(2, 0, psC, "small")
        emit_oproj(2, 1, psD, "den")
        # keep the PE p-state hot while the final AllGather chain is in
        # flight: harmless matmuls into the retired scores banks
        for wi in range(19):
            warm = ps2.tile([P, 2 * TBLK], F32, tag="st2", name=f"warm{wi}")
            for half in range(2):
                nc.tensor.matmul(
                    warm[:, half * TBLK:(half + 1) * TBLK],
                    lhsT=w_sb["wq"][:, 0:P], rhs=w_sb["wq"][:, 0:TBLK],
                    start=True, stop=True,
                )
        late = (1, 3, 5, 7, 0, 2, 4, 6)
        emit_oproj(3, 0, psC, "small", korder=late, split_out=True)
        emit_oproj(3, 1, psD, "den", korder=late, split_out=True)

    nc.compile()
    return nc


_NC_CACHE = {}


def _get_nc(with_collective=True):
    key = with_collective
    if key not in _NC_CACHE:
        _NC_CACHE[key] = build_nc(with_collective)
    return _NC_CACHE[key]


def make_in_maps(x, Wq, Wk, Wv, Wo, bo):
    tri = np.ascontiguousarray(np.triu(np.ones((P, P), np.float32))).astype(BF)
    onescol = np.ones((P, 1), dtype=BF)
    in_maps = []
    for c in range(N_CORES):
        b, g = c // GROUPS, c % GROUPS
        hs_ = slice(g * HPG, (g + 1) * HPG)
        bo_sl = bo[g * ES:(g + 1) * ES].astype(np.float32)
        in_maps.append({
            "xT": np.ascontiguousarray(x[b].T).astype(BF),
            "wq": np.ascontiguousarray(
                Wq[hs_].transpose(1, 0, 2).reshape(C, HD)).astype(BF),
            "wk": np.ascontiguousarray(
                Wk[hs_].transpose(1, 0, 2).reshape(C, HD)).astype(BF),
            "wv": np.ascontiguousarray(
                Wv[hs_].transpose(1, 0, 2).reshape(C, HD)).astype(BF),
            "wo": np.ascontiguousarray(Wo[:, g * ES:(g + 1) * ES]).astype(BF),
            "bo2": np.ascontiguousarray(bo_sl.reshape(2, P).T),
            "trineg": tri,
            "onescol": onescol,
        })
    return in_maps


def kernel(x, Wq, Wk, Wv, Wo, bo):
    x = np.asarray(x, dtype=np.float32)
    Wq = np.asarray(Wq, dtype=np.float32)
    Wk = np.asarray(Wk, dtype=np.float32)
    Wv = np.asarray(Wv, dtype=np.float32)
    Wo = np.asarray(Wo, dtype=np.float32)
    bo = np.asarray(bo, dtype=np.float32)

    nc = _get_nc(with_collective=True)
    in_maps = make_in_maps(x, Wq, Wk, Wv, Wo, bo)
    res = run_bass_kernel_spmd(nc, in_maps, core_ids=list(range(N_CORES)))

    out = np.empty((B, T, E), dtype=np.float32)
    for c in range(N_CORES):
        b, g = c // GROUPS, c % GROUPS
        out[b, :, g * ES:(g + 1) * ES] = res.results[c]["outT"].T
    return out


# revision 8
# speedup vs baseline: 1.3203x; 1.0011x over previous
"""Multi-head causal self-attention on 8 TRN2 NeuronCores — v2 (bf16).

Problem (nn_MultiHeadAttention): B=2, T=2048, C=1024, H=16 heads, hs=64.

Sharding: core c -> (batch b = c//4, head-group g = c%4, 4 heads each).
AllGather of normalized attention outputs across the 4 cores of a batch
(replica groups [0-3], [4-7]); each core computes a disjoint 256-column
slice of the output projection, TRANSPOSED (out^T [256, 2048]); host
transposes + concatenates.

v2 design (cost-model-driven):
- All matmul I/O in bf16 (1 cycle/row at any size; halves SBUF/DMA bytes).
  PSUM accumulation stays f32; rel-err budget 2e-2 >> bf16 noise.
- Scores S^T [s,q] per head pair, rhs q-range trimmed to the causal part.
  Diagonal s-tiles use ONE strided-AP exp (both heads, [128, 2, 512-ka])
  and ONE strided triu mask multiply on DVE — halving the per-op fixed
  overheads on the ACT pacing chain (~6 us).
- AV in [q,d] layout: att[q,h] accumulated per 128-q-subtile with
  lhsT = pt-slice (cost 64/matmul instead of 512). Softmax denominator via
  N=1 matmuls (rhs=ones) into a separate PSUM bank. Normalization becomes a
  reciprocal [128,8] + stride-0-broadcast multiplies (cheap DVE ops).
- att^T for the collective produced by XBAR DMA transpose (no PE/PSUM).
- Output projection computes out^T [e,t] (lhsT=Wo tile, rhs=AllGathered
  att^T), so the bias is per-partition and folds into the PSUM->SBUF copy.
- PSUM bank discipline: matmul start=True zeroes a whole 2KB bank, so each
  bank gets exactly one start and slice-groups accumulate with start=False.
  Banks: scores ping-pong 2x[128,1024] (4), att accum 2x[128,512] (2),
  denominators [128,512] (1), shared small pool (1) = 8.
- DMAs batched (one per weight matrix, per x row-tile, per lh gather) to
  keep the serial HWDGE setup (~630ns each) off the critical path.
"""

import numpy as np
import ml_dtypes
from contextlib import ExitStack

import concourse.bass as bass
import concourse.mybir as mybir
import concourse.tile as tile
from concourse import bacc
from concourse.bass_utils import run_bass_kernel_spmd

F32 = mybir.dt.float32
BF16 = mybir.dt.bfloat16
BF = ml_dtypes.bfloat16
EXP = mybir.ActivationFunctionType.Exp
MULT = mybir.AluOpType.mult

N_CORES = 8
B = 2
T = 2048
C = 1024
NH = 16
HS = 64
E = 1024
GROUPS = 4
HPG = NH // GROUPS   # 4 heads per core
ES = E // GROUPS     # 256 output columns per core
HD = HPG * HS        # 256 local attention-output rows

P = 128
TBLK = 512
NTB = T // TBLK      # 4
NCT = C // P         # 8
NST = T // P         # 16

REPLICA_GROUPS = [[0, 1, 2, 3], [4, 5, 6, 7]]


def build_nc(with_collective=True):
    nc = bacc.Bacc(
        "TRN2", target_bir_lowering=False, debug=False, num_devices=N_CORES
    )

    xT = nc.dram_tensor("xT", [C, T], BF16, kind="ExternalInput").ap()
    wq = nc.dram_tensor("wq", [C, HD], BF16, kind="ExternalInput").ap()
    wk = nc.dram_tensor("wk", [C, HD], BF16, kind="ExternalInput").ap()
    wv = nc.dram_tensor("wv", [C, HD], BF16, kind="ExternalInput").ap()
    wo = nc.dram_tensor("wo", [E, ES], BF16, kind="ExternalInput").ap()
    bo2 = nc.dram_tensor("bo2", [P, 2], F32, kind="ExternalInput").ap()
    trineg = nc.dram_tensor("trineg", [P, P], BF16, kind="ExternalInput").ap()
    onescol = nc.dram_tensor("onescol", [P, 1], BF16, kind="ExternalInput").ap()
    outT = nc.dram_tensor("outT", [ES, T], F32, kind="ExternalOutput").ap()

    with tile.TileContext(nc) as tc, ExitStack() as ctx:
        wp = ctx.enter_context(tc.tile_pool(name="wp", bufs=1))
        xp = ctx.enter_context(tc.tile_pool(name="xp", bufs=1))
        qkp = ctx.enter_context(tc.tile_pool(name="qkp", bufs=1))
        vp = ctx.enter_context(tc.tile_pool(name="vp", bufs=1))
        ptp = ctx.enter_context(tc.tile_pool(name="ptp", bufs=8))
        asp = ctx.enter_context(tc.tile_pool(name="asp", bufs=8))
        atp = ctx.enter_context(tc.tile_pool(name="atp", bufs=4))
        lhp = ctx.enter_context(tc.tile_pool(name="lhp", bufs=6))
        rp = ctx.enter_context(tc.tile_pool(name="rp", bufs=8))
        osp = ctx.enter_context(tc.tile_pool(name="osp", bufs=4))
        # PSUM: 8 banks. ps2 = scores ping-pong (2x2 banks), psA = att
        # accumulators (2x1), psD = denominators (1), psC = small shared (1).
        ps2 = ctx.enter_context(tc.tile_pool(name="ps2", bufs=2, space="PSUM"))
        psA = ctx.enter_context(tc.tile_pool(name="psA", bufs=1, space="PSUM"))
        psD = ctx.enter_context(tc.tile_pool(name="psD", bufs=1, space="PSUM"))
        psC = ctx.enter_context(tc.tile_pool(name="psC", bufs=1, space="PSUM"))
        dramp = ctx.enter_context(tc.tile_pool(name="dramp", bufs=1, space="DRAM"))

        # ---- constants (tiles; DMAs issued after the hot-path loads) ----
        tri_sb = wp.tile([P, P], BF16, tag="tri")
        ones_sb = wp.tile([P, 1], BF16, tag="ones")
        bo_sb = wp.tile([P, 2], F32, tag="bo")

        def load_consts():
            nc.sync.dma_start(tri_sb[:], trineg[:])
            nc.sync.dma_start(ones_sb[:], onescol[:])

        # ---- big weight tiles, one DMA each: [128, 8*width] ----
        w_sb = {}

        def load_w(name, dram, width):
            t_ = wp.tile([P, NCT * width], BF16, tag=name)
            nc.sync.dma_start(
                t_[:].rearrange("p (c e) -> p c e", c=NCT),
                dram.rearrange("(c p) e -> p c e", p=P),
            )
            w_sb[name] = t_

        # ---- x row-tiles: tb0 chunk first (unblocks stage-1 fast), rest
        # loaded in a second wave ----
        x_sb = []

        def load_x_tb0(ci):
            t_ = xp.tile([P, T], BF16, tag=f"x{ci}", name=f"x{ci}")
            nc.sync.dma_start(t_[:, 0:TBLK], xT[ci * P:(ci + 1) * P, 0:TBLK])
            x_sb.append(t_)

        def load_x_rest(ci):
            nc.sync.dma_start(
                x_sb[ci][:, TBLK:T], xT[ci * P:(ci + 1) * P, TBLK:T])

        # merged Q^T/K^T per head pair: col = tb*1024 + which*512 + t_local
        qkt = [qkp.tile([P, 2 * T], BF16, tag=f"qk{p_}", name=f"qk{p_}")
               for p_ in range(2)]

        def qt_slice(pr, hh, t0, tn):
            tb, tl = t0 // TBLK, t0 % TBLK
            base = tb * 1024 + tl
            return qkt[pr][hh * HS:(hh + 1) * HS, base:base + tn]

        def kt_slice(pr, hh, s0, sn):
            tb, sl = s0 // TBLK, s0 % TBLK
            base = tb * 1024 + TBLK + sl
            return qkt[pr][hh * HS:(hh + 1) * HS, base:base + sn]

        v_sb = [vp.tile([P, HD], BF16, tag=f"v{st}", name=f"v{st}")
                for st in range(NST)]

        # ---------------- stage-1 pieces ----------------
        def emit_qk_proj(tb, pr, which, pool=None, ptag="small"):
            pool = pool or psC
            ps_ = pool.tile([P, TBLK], F32, tag=ptag,
                            name=f"qkps{tb}_{pr}_{which}")
            wn = "wq" if which == 0 else "wk"
            for ci in range(NCT):
                nc.tensor.matmul(
                    ps_[:],
                    lhsT=w_sb[wn][:, ci * HD + pr * P:ci * HD + (pr + 1) * P],
                    rhs=x_sb[ci][:, tb * TBLK:(tb + 1) * TBLK],
                    start=(ci == 0), stop=(ci == NCT - 1),
                )
            base = tb * 1024 + which * TBLK
            nc.vector.tensor_copy(qkt[pr][:, base:base + TBLK], ps_[:])

        def emit_v_proj(st, pool=None, ptag="small"):
            pool = pool or psC
            ps_ = pool.tile([P, TBLK], F32, tag=ptag, name=f"vps{st}")
            for ci in range(NCT):
                nc.tensor.matmul(
                    ps_[:, 0:HD],
                    lhsT=x_sb[ci][:, st * P:(st + 1) * P],
                    rhs=w_sb["wv"][:, ci * HD:(ci + 1) * HD],
                    start=(ci == 0), stop=(ci == NCT - 1),
                )
            nc.vector.tensor_copy(v_sb[st][:], ps_[:, 0:HD])

        def qk_chunks(tb):
            return [lambda tb=tb, pr=pr, w=w: emit_qk_proj(tb, pr, w)
                    for pr in range(2) for w in range(2)]

        def v_chunks(tb):
            return [lambda st=st: emit_v_proj(st)
                    for st in range(4 * tb, 4 * tb + 4)]

        # ------- stage-2: s-loop of one head pair of one t-block ------
        att_of = {}   # qb -> [A01, A23] psum tiles [128, 512]
        den_of = {}   # qb -> psum tile [128, 512] (cols pr*8 + sub*2 + hh)

        def new_qb_psum(qb):
            att_of[qb] = [
                psA.tile([P, 4 * P], F32, tag=f"att{b_}", name=f"att{qb}_{b_}")
                for b_ in range(2)
            ]
            den_of[qb] = psD.tile([P, TBLK], F32, tag="den", name=f"den{qb}")

        def emit_headpair(qb, pr):
            """Scores + exp + AV/denominator accumulation for heads
            (2pr, 2pr+1). Yields once per s-tile for the filler driver."""
            t0 = qb * TBLK
            ns = 4 * (qb + 1)
            att, den = att_of[qb], den_of[qb]
            for si in range(ns):
                diag = si * P >= t0
                ka = si * P - t0 if diag else 0
                stp = ps2.tile([P, 2 * TBLK], F32, tag="st2",
                               name=f"st{qb}_{pr}_{si}")
                for hh in range(2):
                    c0 = hh * TBLK + ka
                    nc.tensor.matmul(
                        stp[:, c0:(hh + 1) * TBLK],
                        lhsT=kt_slice(pr, hh, si * P, P),
                        rhs=qt_slice(pr, hh, t0 + ka, TBLK - ka),
                        start=True, stop=True,
                    )
                pt = ptp.tile([P, 2 * TBLK], BF16, tag="pt",
                              name=f"pt{qb}_{pr}_{si}")
                if diag:
                    # one strided exp + one strided mask over both heads:
                    # halves the per-op fixed overheads on the pacing chain
                    sv = stp[:].rearrange("p (h q) -> p h q", h=2)[:, :, ka:TBLK]
                    pv = pt[:].rearrange("p (h q) -> p h q", h=2)[:, :, ka:TBLK]
                    nc.scalar.activation(pv, sv, EXP, scale=0.125)
                    pm = pt[:].rearrange("p (h q) -> p h q", h=2)[:, :, ka:ka + P]
                    tri2 = (tri_sb[:].rearrange("p (h q) -> p h q", h=1)
                            .broadcast_to([P, 2, P]))
                    nc.vector.tensor_mul(pm, pm, tri2)
                else:
                    nc.scalar.activation(pt[:], stp[:], EXP, scale=0.125)
                sub0 = ka // P
                for sub in range(sub0, 4):
                    last = si == 4 * qb + sub
                    for hh in range(2):
                        h = 2 * pr + hh
                        first = pr == 0 and hh == 0 and si == 0 and sub % 2 == 0
                        pslice = pt[:, hh * TBLK + sub * P:
                                    hh * TBLK + (sub + 1) * P]
                        nc.tensor.matmul(
                            att[sub // 2][:, (sub % 2) * HD + h * HS:
                                          (sub % 2) * HD + (h + 1) * HS],
                            lhsT=pslice, rhs=v_sb[si][:, h * HS:(h + 1) * HS],
                            start=first, stop=last, skip_group_check=True,
                        )
                        dfirst = pr == 0 and hh == 0 and si == 0 and sub == 0
                        dcol = pr * 8 + sub * 2 + hh
                        nc.tensor.matmul(
                            den[:, dcol:dcol + 1],
                            lhsT=pslice, rhs=ones_sb[:],
                            start=dfirst, stop=last, skip_group_check=True,
                        )
                # normalize q-subtiles whose denominators just completed
                # (all but the last, which emit_finish_pair handles)
                if si >= 4 * qb and si - 4 * qb < 3:
                    emit_norm_sub(qb, pr, si - 4 * qb)
                yield

        # ------- normalize (per-sub, early) + AllGather for one pair -------
        # att stays in [q, d] layout end-to-end on this side; the transpose
        # to [d, t] happens in the lh load from DRAM via the XBAR.
        lh_of = {}
        asb_of = {}

        def emit_norm_sub(qb, pr, sub):
            """Reciprocal + normalize one q-subtile as soon as its
            denominators are complete (after s-tile si = 4qb+sub)."""
            att, den = att_of[qb], den_of[qb]
            if sub == 0:
                asb_of[(qb, pr)] = asp.tile(
                    [P, 4 * P], BF16, tag=f"asb{pr}", name=f"asb{qb}_{pr}")
            rec = rp.tile([P, 2], F32, tag="rec", name=f"rec{qb}_{pr}_{sub}")
            dcol = pr * 8 + sub * 2
            nc.vector.reciprocal(rec[:], den[:, dcol:dcol + 2])
            in0 = att[sub // 2][:, (sub % 2) * HD + pr * P:
                                (sub % 2) * HD + (pr + 1) * P]
            nc.vector.tensor_tensor(
                asb_of[(qb, pr)][:, sub * P:(sub + 1) * P]
                .rearrange("p (h d) -> p h d", h=2),
                in0.rearrange("p (h d) -> p h d", h=2),
                rec[:].broadcast_to([P, 2, HS]),
                MULT,
            )

        def emit_finish_pair(qb, pr):
            emit_norm_sub(qb, pr, 3)
            a_sb = asb_of[(qb, pr)]
            # AllGather this pair's att [512 t, 128 d] (t-major rows)
            ag_in = dramp.tile([4 * P, P], BF16, tag=f"agin{qb}_{pr}")
            nc.sync.dma_start(
                ag_in[:].rearrange("(s q) d -> q s d", q=P),
                a_sb[:].rearrange("q (s d) -> q s d", s=4),
            )
            ag_out = dramp.tile([GROUPS * 4 * P, P], BF16,
                                tag=f"agout{qb}_{pr}")
            if with_collective:
                nc.gpsimd.collective_compute(
                    "AllGather",
                    mybir.AluOpType.bypass,
                    replica_groups=REPLICA_GROUPS,
                    ins=[ag_in[:].opt()],
                    outs=[ag_out[:].opt()],
                )
            else:  # timing/sim variant: fake the AG with local DMA copies
                nc.sync.dma_start(
                    ag_out[:].rearrange("(g t) d -> t g d", t=4 * P),
                    ag_in[:].rearrange("(g t) d -> t g d", g=1)
                    .broadcast_to([4 * P, GROUPS, P]),
                )
            # transpose-load the gathered chunks two groups at a time:
            # lh half = [128 d, 1024 (g-major t)]
            lhs = []
            for gh in range(2):
                lh_t = lhp.tile([P, 2 * TBLK], BF16, tag=f"lh{pr}_{gh}",
                                name=f"lh{qb}_{pr}_{gh}")
                nc.sync.dma_start_transpose(
                    lh_t[:], ag_out[gh * 8 * P:(gh + 1) * 8 * P, :])
                lhs.append(lh_t)
            lh_of[(qb, pr)] = [
                lhs[g // 2][:, (g % 2) * TBLK:(g % 2 + 1) * TBLK]
                for g in range(GROUPS)
            ]

        # ---------------- stage-3: out^T projection ----------------
        def emit_oproj(qb, et, pool=None, tag="small",
                       korder=(0, 2, 4, 6, 1, 3, 5, 7), split_out=False):
            pool = pool or psC
            op = pool.tile([P, TBLK], F32, tag=tag, name=f"op{qb}_{et}")
            for j, k in enumerate(korder):
                nc.tensor.matmul(
                    op[:],
                    lhsT=w_sb["wo"][:, k * ES + et * P:k * ES + (et + 1) * P],
                    rhs=lh_of[(qb, k % 2)][k // 2],
                    start=(j == 0), stop=(j == NCT - 1),
                )
            o_sb = osp.tile([P, TBLK], F32, tag="osb", name=f"osb{qb}_{et}")
            # asymmetric split: big piece first, small final piece so the
            # last store's latency chain is short
            cuts = (0, 384, TBLK) if split_out else (0, TBLK)
            for h in range(len(cuts) - 1):
                sl = slice(cuts[h], cuts[h + 1])
                nc.vector.tensor_scalar_add(
                    o_sb[:, sl], op[:, sl], bo_sb[:, et:et + 1])
                nc.sync.dma_start(
                    outT[et * P:(et + 1) * P,
                         qb * TBLK + sl.start:qb * TBLK + sl.stop],
                    o_sb[:, sl])

        def oproj_fillers(qb):
            return [lambda et=et, q=qb: emit_oproj(q, et) for et in range(2)]

        # ---------------- emission schedule ----------------
        load_w("wq", wq, HD)
        load_w("wk", wk, HD)
        for ci in range(NCT):
            load_x_tb0(ci)
        load_w("wv", wv, HD)
        load_consts()
        # startup chunks spread across the (still free) PSUM banks so they
        # don't serialize on the single shared bank
        emit_qk_proj(0, 0, 0, psC, "small")
        emit_qk_proj(0, 0, 1, psD, "den")
        emit_v_proj(0, psA, "att0")
        emit_v_proj(1, psA, "att1")
        emit_v_proj(2, psC, "small")
        emit_v_proj(3, psD, "den")
        for ci in range(NCT):
            load_x_rest(ci)
        load_w("wo", wo, ES)
        nc.sync.dma_start(bo_sb[:], bo2[:])

        def drive_pair(qb, pr, vfill, fillers, stride, off=0):
            ctr = 0
            for _ in emit_headpair(qb, pr):
                ctr += 1
                if vfill and ctr % 2 == 1:
                    vfill.pop(0)()
                elif (fillers and ctr > off
                      and (ctr - off) % stride == 0):
                    fillers.pop(0)()

        # fillers per qb: v for qb's own diagonal on odd units (vfill),
        # stage-1 for qb+1 and delayed out-projections strided (fillers)
        qkp1 = [lambda w=w: emit_qk_proj(0, 1, w) for w in range(2)]
        plan = {
            0: ([], qkp1 + qk_chunks(1)),
            1: (v_chunks(1), qk_chunks(2)),
            2: (v_chunks(2), qk_chunks(3)),
            3: (v_chunks(3), oproj_fillers(0) + oproj_fillers(1)),
        }
        for qb in range(NTB):
            new_qb_psum(qb)
            vfill, fillers = plan[qb]
            vfill, fillers = list(vfill), list(fillers)
            ns = 4 * (qb + 1)
            stride = max(1, (2 * ns) // max(1, len(fillers) + 1))
            drive_pair(qb, 0, vfill, fillers, stride)
            emit_finish_pair(qb, 0)
            drive_pair(qb, 1, vfill, fillers, stride)
            while vfill:
                vfill.pop(0)()
            while fillers:
                fillers.pop(0)()
            emit_finish_pair(qb, 1)

        # tail: out-projections of qb2 (its lh landed long ago — keeps PE
        # busy and hot while qb3-pair1's AllGather chain is in flight),
        # then qb3's. The retired den bank doubles as a second accumulator
        # so the two e-tiles run in parallel instead of serializing on psC.
        emit_oproj(2, 0, psC, "small")
        emit_oproj(2, 1, psD, "den")
        # keep the PE p-state hot while the final AllGather chain is in
        # flight: harmless matmuls into the retired scores banks
        for wi in range(19):
            warm = ps2.tile([P, 2 * TBLK], F32, tag="st2", name=f"warm{wi}")
            for half in range(2):
                nc.tensor.matmul(
                    warm[:, half * TBLK:(half + 1) * TBLK],
                    lhsT=w_sb["wq"][:, 0:P], rhs=w_sb["wq"][:, 0:TBLK],
                    start=True, stop=True,
                )
        late = (1, 3, 5, 7, 0, 2, 4, 6)
        emit_oproj(3, 0, psC, "small", korder=late, split_out=True)
        emit_oproj(3, 1, psD, "den", korder=late, split_out=True)

    nc.compile()
    return nc


_NC_CACHE = {}


def _get_nc(with_collective=True):
    key = with_collective
    if key not in _NC_CACHE:
        _NC_CACHE[key] = build_nc(with_collective)
    return _NC_CACHE[key]


def make_in_maps(x, Wq, Wk, Wv, Wo, bo):
    tri = np.ascontiguousarray(np.triu(np.ones((P, P), np.float32))).astype(BF)
    onescol = np.ones((P, 1), dtype=BF)
    in_maps = []
    for c in range(N_CORES):
        b, g = c // GROUPS, c % GROUPS
        hs_ = slice(g * HPG, (g + 1) * HPG)
        bo_sl = bo[g * ES:(g + 1) * ES].astype(np.float32)
        in_maps.append({
            "xT": np.ascontiguousarray(x[b].T).astype(BF),
            "wq": np.ascontiguousarray(
                Wq[hs_].transpose(1, 0, 2).reshape(C, HD)).astype(BF),
            "wk": np.ascontiguousarray(
                Wk[hs_].transpose(1, 0, 2).reshape(C, HD)).astype(BF),
            "wv": np.ascontiguousarray(
                Wv[hs_].transpose(1, 0, 2).reshape(C, HD)).astype(BF),
            "wo": np.ascontiguousarray(Wo[:, g * ES:(g + 1) * ES]).astype(BF),
            "bo2": np.ascontiguousarray(bo_sl.reshape(2, P).T),
            "trineg": tri,
            "onescol": onescol,
        })
    return in_maps


def kernel(x, Wq, Wk, Wv, Wo, bo):
    x = np.asarray(x, dtype=np.float32)
    Wq = np.asarray(Wq, dtype=np.float32)
    Wk = np.asarray(Wk, dtype=np.float32)
    Wv = np.asarray(Wv, dtype=np.float32)
    Wo = np.asarray(Wo, dtype=np.float32)
    bo = np.asarray(bo, dtype=np.float32)

    nc = _get_nc(with_collective=True)
    in_maps = make_in_maps(x, Wq, Wk, Wv, Wo, bo)
    res = run_bass_kernel_spmd(nc, in_maps, core_ids=list(range(N_CORES)))

    out = np.empty((B, T, E), dtype=np.float32)
    for c in range(N_CORES):
        b, g = c // GROUPS, c % GROUPS
        out[b, :, g * ES:(g + 1) * ES] = res.results[c]["outT"].T
    return out


# revision 9
# speedup vs baseline: 1.3206x; 1.0002x over previous
"""Multi-head causal self-attention on 8 TRN2 NeuronCores — v2 (bf16).

Problem (nn_MultiHeadAttention): B=2, T=2048, C=1024, H=16 heads, hs=64.

Sharding: core c -> (batch b = c//4, head-group g = c%4, 4 heads each).
AllGather of normalized attention outputs across the 4 cores of a batch
(replica groups [0-3], [4-7]); each core computes a disjoint 256-column
slice of the output projection, TRANSPOSED (out^T [256, 2048]); host
transposes + concatenates.

v2 design (cost-model-driven):
- All matmul I/O in bf16 (1 cycle/row at any size; halves SBUF/DMA bytes).
  PSUM accumulation stays f32; rel-err budget 2e-2 >> bf16 noise.
- Scores S^T [s,q] per head pair as before, but causal mask applied ON PE:
  an extra accumulate matmul (lhsT=I, rhs=tri(-32768)) onto the diagonal
  128-block, so exp needs no DVE mask multiply.
- AV in [q,d] layout: att[q,h] accumulated per 128-q-subtile with
  lhsT = pt-slice (cost 64/matmul instead of 512). Softmax denominator via
  N=1 matmuls (rhs=ones) into a separate PSUM bank. Normalization becomes a
  reciprocal [128,8] + stride-0-broadcast multiplies (cheap DVE ops).
- att^T for the collective produced by XBAR DMA transpose (no PE/PSUM).
- Output projection computes out^T [e,t] (lhsT=Wo tile, rhs=AllGathered
  att^T), so the bias is per-partition and folds into the PSUM->SBUF copy.
- PSUM bank discipline: matmul start=True zeroes a whole 2KB bank, so each
  bank gets exactly one start and slice-groups accumulate with start=False.
  Banks: scores ping-pong 2x[128,1024] (4), att accum 2x[128,512] (2),
  denominators [128,512] (1), shared small pool (1) = 8.
- DMAs batched (one per weight matrix, per x row-tile, per lh gather) to
  keep the serial HWDGE setup (~630ns each) off the critical path.
"""

import numpy as np
import ml_dtypes
from contextlib import ExitStack

import concourse.bass as bass
import concourse.mybir as mybir
import concourse.tile as tile
from concourse import bacc
from concourse.bass_utils import run_bass_kernel_spmd

F32 = mybir.dt.float32
BF16 = mybir.dt.bfloat16
BF = ml_dtypes.bfloat16
EXP = mybir.ActivationFunctionType.Exp
MULT = mybir.AluOpType.mult

N_CORES = 8
B = 2
T = 2048
C = 1024
NH = 16
HS = 64
E = 1024
GROUPS = 4
HPG = NH // GROUPS   # 4 heads per core
ES = E // GROUPS     # 256 output columns per core
HD = HPG * HS        # 256 local attention-output rows

P = 128
TBLK = 512
NTB = T // TBLK      # 4
NCT = C // P         # 8
NST = T // P         # 16

REPLICA_GROUPS = [[0, 1, 2, 3], [4, 5, 6, 7]]


def build_nc(with_collective=True):
    nc = bacc.Bacc(
        "TRN2", target_bir_lowering=False, debug=False, num_devices=N_CORES
    )

    xT = nc.dram_tensor("xT", [C, T], BF16, kind="ExternalInput").ap()
    wq = nc.dram_tensor("wq", [C, HD], BF16, kind="ExternalInput").ap()
    wk = nc.dram_tensor("wk", [C, HD], BF16, kind="ExternalInput").ap()
    wv = nc.dram_tensor("wv", [C, HD], BF16, kind="ExternalInput").ap()
    wo = nc.dram_tensor("wo", [E, ES], BF16, kind="ExternalInput").ap()
    bo2 = nc.dram_tensor("bo2", [P, 2], F32, kind="ExternalInput").ap()
    trineg = nc.dram_tensor("trineg", [P, P], BF16, kind="ExternalInput").ap()
    onescol = nc.dram_tensor("onescol", [P, 1], BF16, kind="ExternalInput").ap()
    outT = nc.dram_tensor("outT", [ES, T], F32, kind="ExternalOutput").ap()

    with tile.TileContext(nc) as tc, ExitStack() as ctx:
        wp = ctx.enter_context(tc.tile_pool(name="wp", bufs=1))
        xp = ctx.enter_context(tc.tile_pool(name="xp", bufs=1))
        qkp = ctx.enter_context(tc.tile_pool(name="qkp", bufs=1))
        vp = ctx.enter_context(tc.tile_pool(name="vp", bufs=1))
        ptp = ctx.enter_context(tc.tile_pool(name="ptp", bufs=8))
        asp = ctx.enter_context(tc.tile_pool(name="asp", bufs=8))
        atp = ctx.enter_context(tc.tile_pool(name="atp", bufs=4))
        lhp = ctx.enter_context(tc.tile_pool(name="lhp", bufs=6))
        rp = ctx.enter_context(tc.tile_pool(name="rp", bufs=8))
        osp = ctx.enter_context(tc.tile_pool(name="osp", bufs=4))
        # PSUM: 8 banks. ps2 = scores ping-pong (2x2 banks), psA = att
        # accumulators (2x1), psD = denominators (1), psC = small shared (1).
        ps2 = ctx.enter_context(tc.tile_pool(name="ps2", bufs=2, space="PSUM"))
        psA = ctx.enter_context(tc.tile_pool(name="psA", bufs=1, space="PSUM"))
        psD = ctx.enter_context(tc.tile_pool(name="psD", bufs=1, space="PSUM"))
        psC = ctx.enter_context(tc.tile_pool(name="psC", bufs=1, space="PSUM"))
        dramp = ctx.enter_context(tc.tile_pool(name="dramp", bufs=1, space="DRAM"))

        # ---- constants (tiles; DMAs issued after the hot-path loads) ----
        tri_sb = wp.tile([P, P], BF16, tag="tri")
        ones_sb = wp.tile([P, 1], BF16, tag="ones")
        bo_sb = wp.tile([P, 2], F32, tag="bo")

        def load_consts():
            nc.sync.dma_start(tri_sb[:], trineg[:])
            nc.sync.dma_start(ones_sb[:], onescol[:])

        # ---- big weight tiles, one DMA each: [128, 8*width] ----
        w_sb = {}

        def load_w(name, dram, width):
            t_ = wp.tile([P, NCT * width], BF16, tag=name)
            nc.sync.dma_start(
                t_[:].rearrange("p (c e) -> p c e", c=NCT),
                dram.rearrange("(c p) e -> p c e", p=P),
            )
            w_sb[name] = t_

        # ---- x row-tiles: tb0 chunk first (unblocks stage-1 fast), rest
        # loaded in a second wave ----
        x_sb = []

        def load_x_tb0(ci):
            t_ = xp.tile([P, T], BF16, tag=f"x{ci}", name=f"x{ci}")
            nc.sync.dma_start(t_[:, 0:TBLK], xT[ci * P:(ci + 1) * P, 0:TBLK])
            x_sb.append(t_)

        def load_x_rest(ci):
            nc.sync.dma_start(
                x_sb[ci][:, TBLK:T], xT[ci * P:(ci + 1) * P, TBLK:T])

        # merged Q^T/K^T per head pair: col = tb*1024 + which*512 + t_local
        qkt = [qkp.tile([P, 2 * T], BF16, tag=f"qk{p_}", name=f"qk{p_}")
               for p_ in range(2)]

        def qt_slice(pr, hh, t0, tn):
            tb, tl = t0 // TBLK, t0 % TBLK
            base = tb * 1024 + tl
            return qkt[pr][hh * HS:(hh + 1) * HS, base:base + tn]

        def kt_slice(pr, hh, s0, sn):
            tb, sl = s0 // TBLK, s0 % TBLK
            base = tb * 1024 + TBLK + sl
            return qkt[pr][hh * HS:(hh + 1) * HS, base:base + sn]

        v_sb = [vp.tile([P, HD], BF16, tag=f"v{st}", name=f"v{st}")
                for st in range(NST)]

        # ---------------- stage-1 pieces ----------------
        def emit_qk_proj(tb, pr, which, pool=None, ptag="small"):
            pool = pool or psC
            ps_ = pool.tile([P, TBLK], F32, tag=ptag,
                            name=f"qkps{tb}_{pr}_{which}")
            wn = "wq" if which == 0 else "wk"
            for ci in range(NCT):
                nc.tensor.matmul(
                    ps_[:],
                    lhsT=w_sb[wn][:, ci * HD + pr * P:ci * HD + (pr + 1) * P],
                    rhs=x_sb[ci][:, tb * TBLK:(tb + 1) * TBLK],
                    start=(ci == 0), stop=(ci == NCT - 1),
                )
            base = tb * 1024 + which * TBLK
            nc.vector.tensor_copy(qkt[pr][:, base:base + TBLK], ps_[:])

        def emit_v_proj(st, pool=None, ptag="small"):
            pool = pool or psC
            ps_ = pool.tile([P, TBLK], F32, tag=ptag, name=f"vps{st}")
            for ci in range(NCT):
                nc.tensor.matmul(
                    ps_[:, 0:HD],
                    lhsT=x_sb[ci][:, st * P:(st + 1) * P],
                    rhs=w_sb["wv"][:, ci * HD:(ci + 1) * HD],
                    start=(ci == 0), stop=(ci == NCT - 1),
                )
            nc.vector.tensor_copy(v_sb[st][:], ps_[:, 0:HD])

        def qk_chunks(tb):
            return [lambda tb=tb, pr=pr, w=w: emit_qk_proj(tb, pr, w)
                    for pr in range(2) for w in range(2)]

        def v_chunks(tb):
            return [lambda st=st: emit_v_proj(st)
                    for st in range(4 * tb, 4 * tb + 4)]

        # ------- stage-2: s-loop of one head pair of one t-block ------
        att_of = {}   # qb -> [A01, A23] psum tiles [128, 512]
        den_of = {}   # qb -> psum tile [128, 512] (cols pr*8 + sub*2 + hh)

        def new_qb_psum(qb):
            att_of[qb] = [
                psA.tile([P, 4 * P], F32, tag=f"att{b_}", name=f"att{qb}_{b_}")
                for b_ in range(2)
            ]
            den_of[qb] = psD.tile([P, TBLK], F32, tag="den", name=f"den{qb}")

        def emit_headpair(qb, pr):
            """Scores + exp + AV/denominator accumulation for heads
            (2pr, 2pr+1). Yields once per s-tile for the filler driver."""
            t0 = qb * TBLK
            ns = 4 * (qb + 1)
            att, den = att_of[qb], den_of[qb]
            for si in range(ns):
                diag = si * P >= t0
                ka = si * P - t0 if diag else 0
                stp = ps2.tile([P, 2 * TBLK], F32, tag="st2",
                               name=f"st{qb}_{pr}_{si}")
                for hh in range(2):
                    c0 = hh * TBLK + ka
                    nc.tensor.matmul(
                        stp[:, c0:(hh + 1) * TBLK],
                        lhsT=kt_slice(pr, hh, si * P, P),
                        rhs=qt_slice(pr, hh, t0 + ka, TBLK - ka),
                        start=True, stop=True,
                    )
                pt = ptp.tile([P, 2 * TBLK], BF16, tag="pt",
                              name=f"pt{qb}_{pr}_{si}")
                if diag:
                    # one strided exp + one strided mask over both heads:
                    # halves the per-op fixed overheads on the pacing chain
                    sv = stp[:].rearrange("p (h q) -> p h q", h=2)[:, :, ka:TBLK]
                    pv = pt[:].rearrange("p (h q) -> p h q", h=2)[:, :, ka:TBLK]
                    nc.scalar.activation(pv, sv, EXP, scale=0.125)
                    pm = pt[:].rearrange("p (h q) -> p h q", h=2)[:, :, ka:ka + P]
                    tri2 = (tri_sb[:].rearrange("p (h q) -> p h q", h=1)
                            .broadcast_to([P, 2, P]))
                    nc.vector.tensor_mul(pm, pm, tri2)
                else:
                    nc.scalar.activation(pt[:], stp[:], EXP, scale=0.125)
                sub0 = ka // P
                for sub in range(sub0, 4):
                    last = si == 4 * qb + sub
                    for hh in range(2):
                        h = 2 * pr + hh
                        first = pr == 0 and hh == 0 and si == 0 and sub % 2 == 0
                        pslice = pt[:, hh * TBLK + sub * P:
                                    hh * TBLK + (sub + 1) * P]
                        nc.tensor.matmul(
                            att[sub // 2][:, (sub % 2) * HD + h * HS:
                                          (sub % 2) * HD + (h + 1) * HS],
                            lhsT=pslice, rhs=v_sb[si][:, h * HS:(h + 1) * HS],
                            start=first, stop=last, skip_group_check=True,
                        )
                        dfirst = pr == 0 and hh == 0 and si == 0 and sub == 0
                        dcol = pr * 8 + sub * 2 + hh
                        nc.tensor.matmul(
                            den[:, dcol:dcol + 1],
                            lhsT=pslice, rhs=ones_sb[:],
                            start=dfirst, stop=last, skip_group_check=True,
                        )
                # normalize q-subtiles whose denominators just completed
                # (all but the last, which emit_finish_pair handles)
                if si >= 4 * qb and si - 4 * qb < 3:
                    emit_norm_sub(qb, pr, si - 4 * qb)
                yield

        # ------- normalize (per-sub, early) + AllGather for one pair -------
        # att stays in [q, d] layout end-to-end on this side; the transpose
        # to [d, t] happens in the lh load from DRAM via the XBAR.
        lh_of = {}
        asb_of = {}

        def emit_norm_sub(qb, pr, sub):
            """Reciprocal + normalize one q-subtile as soon as its
            denominators are complete (after s-tile si = 4qb+sub)."""
            att, den = att_of[qb], den_of[qb]
            if sub == 0:
                asb_of[(qb, pr)] = asp.tile(
                    [P, 4 * P], BF16, tag=f"asb{pr}", name=f"asb{qb}_{pr}")
            rec = rp.tile([P, 2], F32, tag="rec", name=f"rec{qb}_{pr}_{sub}")
            dcol = pr * 8 + sub * 2
            nc.vector.reciprocal(rec[:], den[:, dcol:dcol + 2])
            in0 = att[sub // 2][:, (sub % 2) * HD + pr * P:
                                (sub % 2) * HD + (pr + 1) * P]
            nc.vector.tensor_tensor(
                asb_of[(qb, pr)][:, sub * P:(sub + 1) * P]
                .rearrange("p (h d) -> p h d", h=2),
                in0.rearrange("p (h d) -> p h d", h=2),
                rec[:].broadcast_to([P, 2, HS]),
                MULT,
            )

        def emit_finish_pair(qb, pr):
            emit_norm_sub(qb, pr, 3)
            a_sb = asb_of[(qb, pr)]
            # AllGather this pair's att [512 t, 128 d] (t-major rows)
            ag_in = dramp.tile([4 * P, P], BF16, tag=f"agin{qb}_{pr}")
            nc.sync.dma_start(
                ag_in[:].rearrange("(s q) d -> q s d", q=P),
                a_sb[:].rearrange("q (s d) -> q s d", s=4),
            )
            ag_out = dramp.tile([GROUPS * 4 * P, P], BF16,
                                tag=f"agout{qb}_{pr}")
            if with_collective:
                nc.gpsimd.collective_compute(
                    "AllGather",
                    mybir.AluOpType.bypass,
                    replica_groups=REPLICA_GROUPS,
                    ins=[ag_in[:].opt()],
                    outs=[ag_out[:].opt()],
                )
            else:  # timing/sim variant: fake the AG with local DMA copies
                nc.sync.dma_start(
                    ag_out[:].rearrange("(g t) d -> t g d", t=4 * P),
                    ag_in[:].rearrange("(g t) d -> t g d", g=1)
                    .broadcast_to([4 * P, GROUPS, P]),
                )
            # transpose-load the gathered chunks two groups at a time:
            # lh half = [128 d, 1024 (g-major t)]
            lhs = []
            for gh in range(2):
                lh_t = lhp.tile([P, 2 * TBLK], BF16, tag=f"lh{pr}_{gh}",
                                name=f"lh{qb}_{pr}_{gh}")
                nc.sync.dma_start_transpose(
                    lh_t[:], ag_out[gh * 8 * P:(gh + 1) * 8 * P, :])
                lhs.append(lh_t)
            lh_of[(qb, pr)] = [
                lhs[g // 2][:, (g % 2) * TBLK:(g % 2 + 1) * TBLK]
                for g in range(GROUPS)
            ]

        # ---------------- stage-3: out^T projection ----------------
        def emit_oproj(qb, et, pool=None, tag="small",
                       korder=(0, 2, 4, 6, 1, 3, 5, 7), split_out=False):
            pool = pool or psC
            op = pool.tile([P, TBLK], F32, tag=tag, name=f"op{qb}_{et}")
            for j, k in enumerate(korder):
                nc.tensor.matmul(
                    op[:],
                    lhsT=w_sb["wo"][:, k * ES + et * P:k * ES + (et + 1) * P],
                    rhs=lh_of[(qb, k % 2)][k // 2],
                    start=(j == 0), stop=(j == NCT - 1),
                )
            o_sb = osp.tile([P, TBLK], F32, tag="osb", name=f"osb{qb}_{et}")
            # asymmetric split: big piece first, small final piece so the
            # last store's latency chain is short
            cuts = (0, 384, TBLK) if split_out else (0, TBLK)
            for h in range(len(cuts) - 1):
                sl = slice(cuts[h], cuts[h + 1])
                nc.vector.tensor_scalar_add(
                    o_sb[:, sl], op[:, sl], bo_sb[:, et:et + 1])
                nc.sync.dma_start(
                    outT[et * P:(et + 1) * P,
                         qb * TBLK + sl.start:qb * TBLK + sl.stop],
                    o_sb[:, sl])

        def oproj_fillers(qb):
            return [lambda et=et, q=qb: emit_oproj(q, et) for et in range(2)]

        # ---------------- emission schedule ----------------
        load_w("wq", wq, HD)
        load_w("wk", wk, HD)
        for ci in range(NCT):
            load_x_tb0(ci)
        load_w("wv", wv, HD)
        load_consts()
        # startup chunks spread across the (still free) PSUM banks so they
        # don't serialize on the single shared bank
        emit_qk_proj(0, 0, 0, psC, "small")
        emit_qk_proj(0, 0, 1, psD, "den")
        emit_v_proj(0, psA, "att0")
        emit_v_proj(1, psA, "att1")
        emit_v_proj(2, psC, "small")
        emit_v_proj(3, psD, "den")
        for ci in range(NCT):
            load_x_rest(ci)
        load_w("wo", wo, ES)
        nc.sync.dma_start(bo_sb[:], bo2[:])

        def drive_pair(qb, pr, vfill, fillers, stride, off=0):
            ctr = 0
            for _ in emit_headpair(qb, pr):
                ctr += 1
                if vfill and ctr % 2 == 1:
                    vfill.pop(0)()
                elif (fillers and ctr > off
                      and (ctr - off) % stride == 0):
                    fillers.pop(0)()

        # fillers per qb: v for qb's own diagonal on odd units (vfill),
        # stage-1 for qb+1 and delayed out-projections strided (fillers)
        qkp1 = [lambda w=w: emit_qk_proj(0, 1, w) for w in range(2)]
        plan = {
            0: ([], qkp1 + qk_chunks(1)),
            1: (v_chunks(1), qk_chunks(2)),
            2: (v_chunks(2), qk_chunks(3)),
            3: (v_chunks(3), oproj_fillers(0) + oproj_fillers(1)),
        }
        for qb in range(NTB):
            new_qb_psum(qb)
            vfill, fillers = plan[qb]
            vfill, fillers = list(vfill), list(fillers)
            ns = 4 * (qb + 1)
            stride = max(1, (2 * ns) // max(1, len(fillers) + 1))
            drive_pair(qb, 0, vfill, fillers, stride)
            emit_finish_pair(qb, 0)
            drive_pair(qb, 1, vfill, fillers, stride)
            while vfill:
                vfill.pop(0)()
            while fillers:
                fillers.pop(0)()
            emit_finish_pair(qb, 1)

        # tail: out-projections of qb2 (its lh landed long ago — keeps PE
        # busy and hot while qb3-pair1's AllGather chain is in flight),
        # then qb3's. The retired den bank doubles as a second accumulator
        # so the two e-tiles run in parallel instead of serializing on psC.
        emit_oproj(2, 0, psC, "small")
        emit_oproj(2, 1, psD, "den")
        # keep the PE p-state hot while the final AllGather chain is in
        # flight: harmless matmuls into the retired scores banks
        for wi in range(19):
            warm = ps2.tile([P, 2 * TBLK], F32, tag="st2", name=f"warm{wi}")
            for qtr in range(4):
                nc.tensor.matmul(
                    warm[:, qtr * 256:(qtr + 1) * 256],
                    lhsT=w_sb["wq"][:, 0:P], rhs=w_sb["wq"][:, 0:256],
                    start=(qtr % 2 == 0), stop=(qtr % 2 == 1),
                )
        late = (1, 3, 5, 7, 0, 2, 4, 6)
        emit_oproj(3, 0, psC, "small", korder=late, split_out=True)
        emit_oproj(3, 1, psD, "den", korder=late, split_out=True)

    nc.compile()
    return nc


_NC_CACHE = {}


def _get_nc(with_collective=True):
    key = with_collective
    if key not in _NC_CACHE:
        _NC_CACHE[key] = build_nc(with_collective)
    return _NC_CACHE[key]


def make_in_maps(x, Wq, Wk, Wv, Wo, bo):
    tri = np.ascontiguousarray(np.triu(np.ones((P, P), np.float32))).astype(BF)
    onescol = np.ones((P, 1), dtype=BF)
    in_maps = []
    for c in range(N_CORES):
        b, g = c // GROUPS, c % GROUPS
        hs_ = slice(g * HPG, (g + 1) * HPG)
        bo_sl = bo[g * ES:(g + 1) * ES].astype(np.float32)
        in_maps.append({
            "xT": np.ascontiguousarray(x[b].T).astype(BF),
            "wq": np.ascontiguousarray(
                Wq[hs_].transpose(1, 0, 2).reshape(C, HD)).astype(BF),
            "wk": np.ascontiguousarray(
                Wk[hs_].transpose(1, 0, 2).reshape(C, HD)).astype(BF),
            "wv": np.ascontiguousarray(
                Wv[hs_].transpose(1, 0, 2).reshape(C, HD)).astype(BF),
            "wo": np.ascontiguousarray(Wo[:, g * ES:(g + 1) * ES]).astype(BF),
            "bo2": np.ascontiguousarray(bo_sl.reshape(2, P).T),
            "trineg": tri,
            "onescol": onescol,
        })
    return in_maps


def kernel(x, Wq, Wk, Wv, Wo, bo):
    x = np.asarray(x, dtype=np.float32)
    Wq = np.asarray(Wq, dtype=np.float32)
    Wk = np.asarray(Wk, dtype=np.float32)
    Wv = np.asarray(Wv, dtype=np.float32)
    Wo = np.asarray(Wo, dtype=np.float32)
    bo = np.asarray(bo, dtype=np.float32)

    nc = _get_nc(with_collective=True)
    in_maps = make_in_maps(x, Wq, Wk, Wv, Wo, bo)
    res = run_bass_kernel_spmd(nc, in_maps, core_ids=list(range(N_CORES)))

    out = np.empty((B, T, E), dtype=np.float32)
    for c in range(N_CORES):
        b, g = c // GROUPS, c % GROUPS
        out[b, :, g * ES:(g + 1) * ES] = res.results[c]["outT"].T
    return out
